# revision 1
# baseline (speedup 1.0000x reference)
"""CrossAttention kernel for 8 Trainium2 NeuronCores.

Problem (hardcoded): B=8, SQ=SK=1024, Q_DIM=2048, KV_DIM=1024, E_DIM=2048,
H=16 heads, HD=128.  out = softmax((X_q Wq^T + bq)(X_k Wk^T + bk)^T / sqrt(HD))
                            @ (X_v Wv^T + bv) @ Wo^T + bo

Sharding: data-parallel over batch — each of the 8 cores computes one batch
element end-to-end; no collectives.

Per-core dataflow (all matmuls bf16 with f32 PSUM accumulation):
  - Host pre-transposes activations/weights so every matmul sees natural
    [K_contraction on partitions] layouts:
      qT[e,s]  = (WqT tiles).T @ xqT      (e on partitions, per-head blocks)
      kT[e,s], vT[e,s] likewise; vT is DMA-transposed into v[s, e] with a
      ones column appended per head (gives softmax denominators for free).
  - Per head: scoresT[sk,sq] = kT_h.T @ qT_h ; P = exp(scoresT) (ACT, bf16)
      oP[sq, 129] = sum_sk P_tile.T @ v[sk, head||ones]   (col 128 = rowsum)
      ao[s, e] = oP[:, :128] * (1/oP[:, 128])             (per-partition)
  - ao is DMA-transposed to aoT[e, s]; out = aoT.T @ WoT (+ bo on host).
"""

import sys

sys.path.insert(0, "/opt/trn_rl_repo")

import numpy as np
import ml_dtypes

import concourse.tile as tile
from concourse import bacc
import concourse.mybir as mybir
from concourse.bass_utils import run_bass_kernel_spmd

F32 = mybir.dt.float32
BF16 = mybir.dt.bfloat16
ACT_IDENT = mybir.ActivationFunctionType.Identity
ACT_EXP = mybir.ActivationFunctionType.Exp

B = 8
S = 1024          # SQ == SK
DQ = 2048         # query input dim
DKV = 1024        # key/value input dim
E = 2048          # embed dim
H = 16            # heads
HD = 128          # head dim
NT_S = S // 128   # 8 seq tiles
NT_E = E // 128   # 16 e tiles (== heads)
NT_DQ = DQ // 128
NT_DKV = DKV // 128
VROW = HD + 1     # head block in v_sb incl. ones column

_CACHED = {}


def _build():
    nc = bacc.Bacc("TRN2", target_bir_lowering=False, debug=False)

    xqT = nc.dram_tensor("xqT", [DQ, S], BF16, kind="ExternalInput")
    xkT = nc.dram_tensor("xkT", [DKV, S], BF16, kind="ExternalInput")
    xvT = nc.dram_tensor("xvT", [DKV, S], BF16, kind="ExternalInput")
    wqT = nc.dram_tensor("wqT", [DQ, E], BF16, kind="ExternalInput")
    wkT = nc.dram_tensor("wkT", [DKV, E], BF16, kind="ExternalInput")
    wvT = nc.dram_tensor("wvT", [DKV, E], BF16, kind="ExternalInput")
    woT = nc.dram_tensor("woT", [E, E], BF16, kind="ExternalInput")
    bq = nc.dram_tensor("bq", [E], F32, kind="ExternalInput")
    bk = nc.dram_tensor("bk", [E], F32, kind="ExternalInput")
    bv = nc.dram_tensor("bv", [E], F32, kind="ExternalInput")
    out = nc.dram_tensor("out", [S, E], F32, kind="ExternalOutput")

    with tile.TileContext(nc) as tc:
        with (
            tc.tile_pool(name="persist", bufs=1) as persist,
            tc.tile_pool(name="wstream", bufs=8) as wstream,
            tc.tile_pool(name="pts", bufs=15) as pts,
            tc.tile_pool(name="outsb", bufs=4) as outsb,
            tc.tile_pool(name="small", bufs=2) as small,
            tc.tile_pool(name="bigps", bufs=4, space="PSUM") as bigps,
        ):
            # ---- load inputs / constants ----
            xq_sb = persist.tile([128, NT_DQ, S], BF16, tag="big_a")
            xk_sb = [pts.tile([128, S], BF16, tag="pt", name=f"xk{_d}")
                     for _d in range(NT_DKV)]
            xv_sb = [pts.tile([128, S], BF16, tag="pt", name=f"xv{_d}")
                     for _d in range(NT_DKV)]
            xqT_r = xqT.rearrange("(t p) s -> p t s", p=128)
            for _d in range(NT_DQ):
                nc.sync.dma_start(out=xq_sb[:, _d, :], in_=xqT_r[:, _d, :])
            xkT_r = xkT.rearrange("(t p) s -> p t s", p=128)
            xvT_r = xvT.rearrange("(t p) s -> p t s", p=128)
            for _d in range(NT_DKV):
                nc.sync.dma_start(out=xk_sb[_d], in_=xkT_r[:, _d, :])
                nc.sync.dma_start(out=xv_sb[_d], in_=xvT_r[:, _d, :])
            bq_sb = persist.tile([128, NT_E], F32, tag="bq")
            bk_sb = persist.tile([128, NT_E], F32, tag="bk")
            bv_sb = persist.tile([128, NT_E], F32, tag="bv")
            nc.sync.dma_start(out=bq_sb, in_=bq.rearrange("(t p) -> p t", p=128))
            nc.sync.dma_start(out=bk_sb, in_=bk.rearrange("(t p) -> p t", p=128))
            nc.sync.dma_start(out=bv_sb, in_=bv.rearrange("(t p) -> p t", p=128))

            qT_sb = persist.tile([128, NT_E, S], BF16, tag="qT")
            kT_sb = persist.tile([128, NT_E, S], BF16, tag="kT")
            vT_sb = persist.tile([128, NT_E, S], BF16, tag="big_b")
            v_sb = persist.tile([128, NT_S, H * VROW], BF16, tag="v")
            # ones columns for the softmax-denominator trick
            nc.vector.memset(
                v_sb.rearrange("p t (h c) -> p t h c", c=VROW)[:, :, :, HD:], 1.0
            )

            # ---- projections: produce qT/kT/vT in [e_partition, s] layout ----
            # out_psum[e128, s1024] accumulated over d tiles; weight blocks of
            # 256 e-columns streamed once; X resident.
            def project(w_dram, x_sb, nt_d, bias_sb, dst_sb, wtag):
                for eb in range(E // 256):
                    ps = [bigps.tile([128, S], F32, tag="ps", name=f"projps{_i}")
                          for _i in range(2)]
                    for d in range(nt_d):
                        wt = wstream.tile([128, 256], BF16, tag="w",
                                          name=f"w_{wtag}_{eb}_{d}")
                        nc.scalar.dma_start(
                            out=wt,
                            in_=w_dram[d * 128:(d + 1) * 128, eb * 256:(eb + 1) * 256],
                        )
                        xd = x_sb[:, d, :] if not isinstance(x_sb, list) \
                            else x_sb[d]
                        for es in range(2):
                            for sc in range(2):
                                nc.tensor.matmul(
                                    ps[es][:, sc * 512:(sc + 1) * 512],
                                    wt[:, es * 128:(es + 1) * 128],
                                    xd[:, sc * 512:(sc + 1) * 512],
                                    start=(d == 0),
                                    stop=(d == nt_d - 1),
                                )
                    for es in range(2):
                        et = eb * 2 + es
                        nc.scalar.activation(
                            out=dst_sb[:, et, :],
                            in_=ps[es],
                            func=ACT_IDENT,
                            bias=bias_sb[:, et:et + 1],
                            scale=1.0,
                        )

            project(wqT, xq_sb, NT_DQ, bq_sb, qT_sb, "wq")
            project(wkT, xk_sb, NT_DKV, bk_sb, kT_sb, "wk")
            project(wvT, xv_sb, NT_DKV, bv_sb, vT_sb, "wv")

            # vT[e,s] -> v[s, head-block] via DMA transpose (per head/e-tile)
            for h in range(H):
                vtmp = wstream.tile([128, NT_S, HD], BF16, tag="vtmp",
                                    name=f"vtmp{h}", bufs=2)
                nc.sync.dma_start_transpose(out=vtmp, in_=vT_sb[:, h, :])
                nc.vector.tensor_copy(
                    out=v_sb[:, :, h * VROW:h * VROW + HD], in_=vtmp)

            # ---- attention per head ----
            ao_sb = persist.tile([128, NT_S, E], BF16, tag="big_a")
            for h in range(H):
                pt_t = [pts.tile([128, S], BF16, tag="pt", name=f"pt{h}_{_sk}")
                        for _sk in range(NT_S)]
                for sk in range(NT_S):
                    ssp = bigps.tile([128, S], F32, tag="ps")
                    for sc in range(2):
                        nc.tensor.matmul(
                            ssp[:, sc * 512:(sc + 1) * 512],
                            kT_sb[:, h, sk * 128:(sk + 1) * 128],
                            qT_sb[:, h, sc * 512:(sc + 1) * 512],
                            start=True,
                            stop=True,
                        )
                    nc.scalar.activation(
                        out=pt_t[sk], in_=ssp, func=ACT_EXP, bias=0.0, scale=1.0
                    )
                for sq in range(NT_S):
                    op_t = bigps.tile([128, S], F32, tag="ps", name=f"op{h}_{sq}")
                    op = op_t[:, 0:VROW]
                    for sk in range(NT_S):
                        nc.tensor.matmul(
                            op,
                            pt_t[sk][:, sq * 128:(sq + 1) * 128],
                            v_sb[:, sk, h * VROW:(h + 1) * VROW],
                            start=(sk == 0),
                            stop=(sk == NT_S - 1),
                        )
                    recip = small.tile([128, 1], F32, tag="recip")
                    nc.vector.reciprocal(out=recip, in_=op[:, HD:VROW])
                    nc.vector.tensor_scalar_mul(
                        ao_sb[:, sq, h * 128:(h + 1) * 128], op[:, 0:HD], recip
                    )

            # ---- WoT resident: halves reuse the qT/kT slots (now dead) ----
            wo0_sb = persist.tile([128, NT_E, 1024], BF16, tag="qT", name="wo0")
            wo1_sb = persist.tile([128, NT_E, 1024], BF16, tag="kT", name="wo1")
            woT_r = woT.rearrange("(t p) n -> p t n", p=128)
            for et in range(NT_E):
                nc.scalar.dma_start(out=wo0_sb[:, et, :], in_=woT_r[:, et, 0:1024])
                nc.scalar.dma_start(out=wo1_sb[:, et, :], in_=woT_r[:, et, 1024:2048])
            wo_half = [wo0_sb, wo1_sb]

            # ---- transpose ao[s,e] -> aoT[e,s] ----
            aoT_sb = persist.tile([128, NT_E, S], BF16, tag="big_b")
            for sq in range(NT_S):
                nc.sync.dma_start_transpose(
                    out=aoT_sb[:, :, sq * 128:(sq + 1) * 128],
                    in_=ao_sb[:, sq, :],
                )

            # ---- output projection: out[s, e2] = aoT.T @ WoT ----
            for sg in range(2):
                for ep in range(2):  # 1024-wide e2 column halves
                    pso = [bigps.tile([128, S], F32, tag="ps", name=f"pso{_i}")
                           for _i in range(NT_S // 2)]
                    for et in range(NT_E):
                        for st4 in range(NT_S // 2):
                            st = sg * (NT_S // 2) + st4
                            for ech in range(2):
                                nc.tensor.matmul(
                                    pso[st4][:, ech * 512:(ech + 1) * 512],
                                    aoT_sb[:, et, st * 128:(st + 1) * 128],
                                    wo_half[ep][:, et, ech * 512:(ech + 1) * 512],
                                    start=(et == 0),
                                    stop=(et == NT_E - 1),
                                )
                    for st4 in range(NT_S // 2):
                        st = sg * (NT_S // 2) + st4
                        for ech in range(2):
                            ot = outsb.tile([128, 512], F32, tag="outt",
                                            name=f"ot{sg}_{ep}_{st4}_{ech}")
                            nc.vector.tensor_copy(
                                out=ot, in_=pso[st4][:, ech * 512:(ech + 1) * 512])
                            nc.sync.dma_start(
                                out=out.ap()[st * 128:(st + 1) * 128,
                                             (ep * 1024 + ech * 512):
                                             (ep * 1024 + (ech + 1) * 512)],
                                in_=ot,
                            )

    nc.compile()
    return nc


def _get_nc():
    if "nc" not in _CACHED:
        _CACHED["nc"] = _build()
    return _CACHED["nc"]


def _numpy_reference(query, key, value, attention_mask,
                     Wq, bq, Wk, bk, Wv, bv, Wo, bo):
    # general fallback (only used when attention_mask isn't all ones)
    Bb, SQ, _ = query.shape
    SK = key.shape[1]
    q = query @ Wq.T + bq
    k = key @ Wk.T + bk
    v = value @ Wv.T + bv
    q = q.reshape(Bb, SQ, H, HD).transpose(0, 2, 1, 3)
    k = k.reshape(Bb, SK, H, HD).transpose(0, 2, 1, 3)
    v = v.reshape(Bb, SK, H, HD).transpose(0, 2, 1, 3)
    scores = np.einsum("bhqd,bhkd->bhqk", q, k) * (HD ** -0.5)
    scores = np.where(attention_mask[:, None, :, :] == 0,
                      np.float32(-1e10), scores)
    scores -= scores.max(-1, keepdims=True)
    p = np.exp(scores)
    p /= p.sum(-1, keepdims=True)
    o = np.einsum("bhqk,bhkd->bhqd", p, v)
    o = o.transpose(0, 2, 1, 3).reshape(Bb, SQ, E)
    return (o @ Wo.T + bo).astype(np.float32)


def _prepare_in_maps(inputs):
    query = np.asarray(inputs["query"], dtype=np.float32)
    key = np.asarray(inputs["key"], dtype=np.float32)
    value = np.asarray(inputs["value"], dtype=np.float32)
    Wq = np.asarray(inputs["Wq"], dtype=np.float32)
    bq = np.asarray(inputs["bq"], dtype=np.float32)
    Wk = np.asarray(inputs["Wk"], dtype=np.float32)
    bk = np.asarray(inputs["bk"], dtype=np.float32)
    Wv = np.asarray(inputs["Wv"], dtype=np.float32)
    bv = np.asarray(inputs["bv"], dtype=np.float32)
    Wo = np.asarray(inputs["Wo"], dtype=np.float32)

    scale = np.float32(HD ** -0.5)
    bf = ml_dtypes.bfloat16
    wqT = np.ascontiguousarray((Wq.T * scale).astype(bf))
    wkT = np.ascontiguousarray(Wk.T.astype(bf))
    wvT = np.ascontiguousarray(Wv.T.astype(bf))
    woT = np.ascontiguousarray(Wo.T.astype(bf))
    bq_s = (bq * scale).astype(np.float32)

    in_maps = []
    for b in range(B):
        in_maps.append({
            "xqT": np.ascontiguousarray(query[b].T.astype(bf)),
            "xkT": np.ascontiguousarray(key[b].T.astype(bf)),
            "xvT": np.ascontiguousarray(value[b].T.astype(bf)),
            "wqT": wqT, "wkT": wkT, "wvT": wvT, "woT": woT,
            "bq": bq_s, "bk": bk.astype(np.float32),
            "bv": bv.astype(np.float32),
        })
    return in_maps


def run_on_device(inputs, **spmd_kwargs):
    """Run the bass kernel; returns (out [B,S,E] f32, BassKernelResults)."""
    in_maps = _prepare_in_maps(inputs)
    bo = np.asarray(inputs["bo"], dtype=np.float32)
    res = run_bass_kernel_spmd(_get_nc(), in_maps,
                               core_ids=list(range(B)), **spmd_kwargs)
    out = np.stack([res.results[b]["out"] for b in range(B)], axis=0)
    return (out + bo).astype(np.float32), res


def kernel(**inputs):
    mask = np.asarray(inputs["attention_mask"])
    if not mask.all():
        return _numpy_reference(
            np.asarray(inputs["query"], dtype=np.float32),
            np.asarray(inputs["key"], dtype=np.float32),
            np.asarray(inputs["value"], dtype=np.float32), mask,
            np.asarray(inputs["Wq"], dtype=np.float32),
            np.asarray(inputs["bq"], dtype=np.float32),
            np.asarray(inputs["Wk"], dtype=np.float32),
            np.asarray(inputs["bk"], dtype=np.float32),
            np.asarray(inputs["Wv"], dtype=np.float32),
            np.asarray(inputs["bv"], dtype=np.float32),
            np.asarray(inputs["Wo"], dtype=np.float32),
            np.asarray(inputs["bo"], dtype=np.float32))
    out, _ = run_on_device(inputs)
    return out



# revision 24
# speedup vs baseline: 1.3482x; 1.3482x over previous
"""CrossAttention kernel for 8 Trainium2 NeuronCores.

Problem (hardcoded): B=8, SQ=SK=1024, Q_DIM=2048, KV_DIM=1024, E_DIM=2048,
H=16 heads, HD=128.  out = softmax((X_q Wq^T + bq)(X_k Wk^T + bk)^T / sqrt(HD))
                            @ (X_v Wv^T + bv) @ Wo^T + bo

Sharding: data-parallel over batch — each of the 8 cores computes one batch
element end-to-end; no collectives.

Per-core dataflow (all matmuls bf16, f32 PSUM accumulation), software-
pipelined per head so the scalar engine's exp() hides under the next head's
projections:

  iter h: [scores(h) tiles interleaved with qproj(h+1)/kproj(h+1)/vproj part]
          then PV(h) -> ao_h -> DMA-transpose into aoT[:, h, :].
  - qT/kT produced in [e, s] layout (weight stationary).
  - v produced directly in [s, e] layout (xvT stationary, wv moving) with a
    ones column per head block => softmax denominators ride along as PV
    output column 128.  bv is folded into bo on the host (softmax rows sum
    to 1, so + bv passes through attention exactly).
  - out = aoT.T @ WoT accumulated over e-tiles with Wo streamed from DRAM;
    chunks copied+stored as they complete.
"""

import sys

sys.path.insert(0, "/opt/trn_rl_repo")

import numpy as np
import ml_dtypes

import concourse.tile as tile
from concourse import bacc
import concourse.mybir as mybir
from concourse.bass_utils import run_bass_kernel_spmd

F32 = mybir.dt.float32
BF16 = mybir.dt.bfloat16
ACT_IDENT = mybir.ActivationFunctionType.Identity
ACT_COPY = mybir.ActivationFunctionType.Copy
ACT_EXP = mybir.ActivationFunctionType.Exp

B = 8
S = 1024          # SQ == SK
DQ = 2048         # query input dim
DKV = 1024        # key/value input dim
E = 2048          # embed dim
H = 16            # heads
HD = 128          # head dim
NT_S = S // 128   # 8 seq tiles
NT_E = E // 128   # 16 e tiles (== heads)
NT_DQ = DQ // 128
NT_DKV = DKV // 128
VROW = HD + 1     # head block in v group incl. ones column

_CACHED = {}


def _build():
    nc = bacc.Bacc("TRN2", target_bir_lowering=False, debug=False)

    xqT = nc.dram_tensor("xqT", [DQ, S], BF16, kind="ExternalInput")
    xkT = nc.dram_tensor("xkT", [DKV, S], BF16, kind="ExternalInput")
    xvT = nc.dram_tensor("xvT", [DKV, S], BF16, kind="ExternalInput")
    wqT = nc.dram_tensor("wqT", [DQ, E], BF16, kind="ExternalInput")
    wkT = nc.dram_tensor("wkT", [DKV, E], BF16, kind="ExternalInput")
    wvT = nc.dram_tensor("wvT", [DKV, E], BF16, kind="ExternalInput")
    woT = nc.dram_tensor("woT", [E, E], BF16, kind="ExternalInput")
    bq = nc.dram_tensor("bq", [E], F32, kind="ExternalInput")
    bk = nc.dram_tensor("bk", [E], F32, kind="ExternalInput")
    out = nc.dram_tensor("out", [S, E], F32, kind="ExternalOutput")

    xqT_r = xqT.rearrange("(t p) s -> p t s", p=128)
    xkT_r = xkT.rearrange("(t p) s -> p t s", p=128)
    xvT_r = xvT.rearrange("(t p) s -> p t s", p=128)
    wqT_r = wqT.rearrange("(t p) e -> p t e", p=128)
    wkT_r = wkT.rearrange("(t p) e -> p t e", p=128)
    wvT_r = wvT.rearrange("(t p) e -> p t e", p=128)

    with tile.TileContext(nc) as tc:
        with (
            tc.tile_pool(name="persist", bufs=1) as persist,
            tc.tile_pool(name="qk", bufs=3) as qkp,
            tc.tile_pool(name="v4p", bufs=2) as v4p,
            tc.tile_pool(name="pts", bufs=9) as pts,
            tc.tile_pool(name="aohp", bufs=2) as aohp,
            tc.tile_pool(name="wqp", bufs=2) as wqp,
            tc.tile_pool(name="wkp", bufs=2) as wkp,
            tc.tile_pool(name="wvp", bufs=2) as wvp,
            tc.tile_pool(name="wop", bufs=6) as wop,
            tc.tile_pool(name="wo2p", bufs=3) as wo2p,
            tc.tile_pool(name="outsb", bufs=3) as outsb,
            tc.tile_pool(name="small", bufs=4) as small,
            tc.tile_pool(name="ssps", bufs=2, space="PSUM") as ssps,
            tc.tile_pool(name="paps", bufs=2, space="PSUM") as paps,
            tc.tile_pool(name="opps", bufs=2, space="PSUM") as opps,
        ):
            # ---- resident inputs / constants ----
            bq_sb = persist.tile([128, NT_E], F32, tag="bq")
            bk_sb = persist.tile([128, NT_E], F32, tag="bk")
            nc.gpsimd.dma_start(out=bq_sb, in_=bq.rearrange("(t p) -> p t", p=128))
            nc.gpsimd.dma_start(out=bk_sb, in_=bk.rearrange("(t p) -> p t", p=128))

            xq_sb = persist.tile([128, NT_DQ, S], BF16, tag="xq")
            xk_sb = persist.tile([128, NT_DKV, S], BF16, tag="xk")
            xv_sb = persist.tile([128, NT_DKV, S], BF16, tag="xv")
            aoT_sb = persist.tile([128, NT_E, S], BF16, tag="aoT")

            # weight slices (wq/wk cover 2 heads per slice)
            wq_sl = {}
            wk_sl = {}
            wv_sl = {}

            def load_wqk2(j, eng=None):  # heads 2j, 2j+1
                eng = eng or nc.scalar
                wq_sl[j] = wqp.tile([128, NT_DQ, 256], BF16, tag="wq",
                                    name=f"wq2_{j}")
                eng.dma_start(
                    out=wq_sl[j], in_=wqT_r[:, :, 2 * j * 128:(2 * j + 2) * 128])
                wk_sl[j] = wkp.tile([128, NT_DKV, 256], BF16, tag="wk",
                                    name=f"wk2_{j}")
                eng.dma_start(
                    out=wk_sl[j], in_=wkT_r[:, :, 2 * j * 128:(2 * j + 2) * 128])

            def load_wv(g, eng=None):  # heads 4g..4g+3
                eng = eng or nc.scalar
                wv_sl[g] = wvp.tile([128, NT_DKV, 512], BF16, tag="wv",
                                    name=f"wv_{g}")
                eng.dma_start(
                    out=wv_sl[g], in_=wvT_r[:, :, 4 * g * 128:(4 * g + 4) * 128])

            # Prologue DMAs: ALL on the sync queue, in exact consumption
            # order (the DMA device serves transfers in arrival order, and
            # cross-queue ordering is uncontrolled): kproj needs wk+xk,
            # then vproj g0 needs wv+xv, then qproj needs wq+xq.
            wk_sl[0] = wkp.tile([128, NT_DKV, 256], BF16, tag="wk", name="wk2_0")
            nc.sync.dma_start(out=wk_sl[0], in_=wkT_r[:, :, 0:256])
            for d in range(NT_DKV):
                nc.sync.dma_start(out=xk_sb[:, d, :], in_=xkT_r[:, d, :])
            load_wv(0, eng=nc.sync)
            for d in range(NT_DKV):
                nc.sync.dma_start(out=xv_sb[:, d, :], in_=xvT_r[:, d, :])
            wq_sl[0] = wqp.tile([128, NT_DQ, 256], BF16, tag="wq", name="wq2_0")
            nc.sync.dma_start(out=wq_sl[0], in_=wqT_r[:, :, 0:256])
            for d in range(NT_DQ):
                nc.sync.dma_start(out=xq_sb[:, d, :], in_=xqT_r[:, d, :])
            load_wqk2(1, eng=nc.sync)
            load_wv(1, eng=nc.sync)
            # prefetch first out-projection weight tiles (parked until the
            # epilogue; also throttles the gpsimd wo stream via pool slots)
            wo_pre = []
            for et in range(6):
                wo_t = wop.tile([128, 512], BF16, tag="wo", name=f"wo_0_{et}")
                nc.sync.dma_start(
                    out=wo_t, in_=woT.ap()[et * 128:(et + 1) * 128, 0:512])
                wo_pre.append(wo_t)

            qT = {}
            kT = {}
            v4 = {}
            _CARRY = {}

            def qproj(h):
                """qT[h] [e128, s] <- sum_d wq-block.T @ xq (2 psum chunks)."""
                sl = wq_sl[h // 2]
                hoff = (h % 2) * 128
                qT[h] = qkp.tile([128, S], BF16, tag="qT", name=f"qT_{h}")
                for c in range(2):
                    ps = paps.tile([128, 512], F32, tag="pa", name=f"qps_{h}_{c}")
                    for d in range(NT_DQ):
                        nc.tensor.matmul(
                            ps,
                            sl[:, d, hoff:hoff + 128],
                            xq_sb[:, d, c * 512:(c + 1) * 512],
                            start=(d == 0),
                            stop=(d == NT_DQ - 1),
                        )
                    nc.scalar.activation(
                        out=qT[h][:, c * 512:(c + 1) * 512], in_=ps,
                        func=ACT_IDENT, bias=bq_sb[:, h:h + 1], scale=1.0)

            def kproj(h):
                sl = wk_sl[h // 2]
                hoff = (h % 2) * 128
                kT[h] = qkp.tile([128, S], BF16, tag="kT", name=f"kT_{h}")
                for c in range(2):
                    ps = paps.tile([128, 512], F32, tag="pa", name=f"kps_{h}_{c}")
                    for d in range(NT_DKV):
                        nc.tensor.matmul(
                            ps,
                            sl[:, d, hoff:hoff + 128],
                            xk_sb[:, d, c * 512:(c + 1) * 512],
                            start=(d == 0),
                            stop=(d == NT_DKV - 1),
                        )
                    nc.scalar.activation(
                        out=kT[h][:, c * 512:(c + 1) * 512], in_=ps,
                        func=ACT_IDENT, bias=bk_sb[:, h:h + 1], scale=1.0)

            def v4_alloc(g):
                v4[g] = v4p.tile([128, NT_S, 4 * VROW], BF16, tag="v4",
                                 name=f"v4_{g}")
                nc.vector.memset(
                    v4[g].rearrange("p t (h c) -> p t h c", c=VROW)
                    [:, :, :, HD:], 1.0)

            def vproj_part(g, st0, nst):
                """v4[g][:, st, :] <- (xv st-block).T @ wv-group, s-tiles
                st0..st0+nst-1; heads 4g..4g+3 with ones columns."""
                sl = wv_sl[g]
                for st in range(st0, st0 + nst):
                    ps = paps.tile([128, 512], F32, tag="pa",
                                   name=f"vps_{g}_{st}")
                    for d in range(NT_DKV):
                        nc.tensor.matmul(
                            ps,
                            xv_sb[:, d, st * 128:(st + 1) * 128],
                            sl[:, d, :],
                            start=(d == 0),
                            stop=(d == NT_DKV - 1),
                        )
                    nc.vector.tensor_copy(
                        out=v4[g][:, st, :].rearrange(
                            "p (h c) -> p h c", c=VROW)[:, :, 0:HD],
                        in_=ps.rearrange("p (h c) -> p h c", c=HD),
                    )

            def scores_pair(h, pt_t, s0):
                """two score tiles s0, s0+1: scoresT[sk,sq] -> exp -> pt."""
                for sk in (s0, s0 + 1):
                    ss = ssps.tile([128, S], F32, tag="ss",
                                   name=f"ss_{h}_{sk}")
                    for c in range(2):
                        nc.tensor.matmul(
                            ss[:, c * 512:(c + 1) * 512],
                            kT[h][:, sk * 128:(sk + 1) * 128],
                            qT[h][:, c * 512:(c + 1) * 512],
                            start=True,
                            stop=True,
                        )
                    nc.scalar.activation(
                        out=pt_t[sk], in_=ss, func=ACT_EXP, bias=0.0, scale=1.0)

            def pv(h, pt_t):
                g, hig = h // 4, h % 4
                ao_h = aohp.tile([128, NT_S, HD], BF16, tag="aoh",
                                 name=f"aoh_{h}")
                for st in range(NT_S):
                    op = opps.tile([128, 512], F32, tag="op",
                                   name=f"op_{h}_{st}")
                    for sk in range(NT_S):
                        nc.tensor.matmul(
                            op[:, 0:VROW],
                            pt_t[sk][:, st * 128:(st + 1) * 128],
                            v4[g][:, sk, hig * VROW:(hig + 1) * VROW],
                            start=(sk == 0),
                            stop=(sk == NT_S - 1),
                        )
                    rec = small.tile([128, 1], F32, tag="rec",
                                     name=f"rec_{h}_{st}")
                    nc.vector.reciprocal(out=rec, in_=op[:, HD:VROW])
                    nc.scalar.activation(
                        out=ao_h[:, st, :], in_=op[:, 0:HD],
                        func=ACT_COPY, bias=0.0, scale=rec)
                nc.sync.dma_start_transpose(
                    out=aoT_sb[:, h, :].rearrange("p (t c) -> p t c", c=128),
                    in_=ao_h)

            # ---- prologue compute, d-outer interleaved so PE consumption
            # rate (4 mms per d-tile) stays behind the DMA arrival rate ----
            def prologue_qk2(proj_wsl, proj_x, nt_d, bias_sb, dst, nm):
                dst[0] = qkp.tile([128, S], BF16, tag=nm, name=f"{nm}_0")
                dst[1] = qkp.tile([128, S], BF16, tag=nm, name=f"{nm}_1")
                ps0 = [paps.tile([128, 512], F32, tag="pa",
                                 name=f"{nm}p0_{c}") for c in range(2)]
                sst = ssps.tile([128, S], F32, tag="ss", name=f"{nm}p1")
                ps1 = [sst[:, 0:512], sst[:, 512:1024]]
                for d in range(nt_d):
                    for hh, pss in ((0, ps0), (1, ps1)):
                        for c in range(2):
                            nc.tensor.matmul(
                                pss[c],
                                proj_wsl[:, d, hh * 128:(hh + 1) * 128],
                                proj_x[:, d, c * 512:(c + 1) * 512],
                                start=(d == 0),
                                stop=(d == nt_d - 1),
                            )
                for hh, pss in ((0, ps0), (1, ps1)):
                    for c in range(2):
                        nc.scalar.activation(
                            out=dst[hh][:, c * 512:(c + 1) * 512], in_=pss[c],
                            func=ACT_IDENT, bias=bias_sb[:, hh:hh + 1],
                            scale=1.0)

            prologue_qk2(wk_sl[0], xk_sb, NT_DKV, bk_sb, kT, "kT")
            v4_alloc(0)
            # vproj g0: two rounds of 4 s-tiles, d-outer (2 pa + 1 ss tile)
            for rnd in range(2):
                pv_ps = [paps.tile([128, 512], F32, tag="pa",
                                   name=f"vp{rnd}_{i}") for i in range(2)]
                sst = ssps.tile([128, S], F32, tag="ss", name=f"vp{rnd}ss")
                pv_ps.append(sst[:, 0:512])
                pv_ps.append(sst[:, 512:1024])
                for d in range(NT_DKV):
                    for i in range(4):
                        st = rnd * 4 + i
                        nc.tensor.matmul(
                            pv_ps[i],
                            xv_sb[:, d, st * 128:(st + 1) * 128],
                            wv_sl[0][:, d, :],
                            start=(d == 0),
                            stop=(d == NT_DKV - 1),
                        )
                for i in range(4):
                    st = rnd * 4 + i
                    nc.vector.tensor_copy(
                        out=v4[0][:, st, :].rearrange(
                            "p (h c) -> p h c", c=VROW)[:, :, 0:HD],
                        in_=pv_ps[i].rearrange("p (h c) -> p h c", c=HD),
                    )
            prologue_qk2(wq_sl[0], xq_sb, NT_DQ, bq_sb, qT, "qT")

            # ---- main loop over heads; iter h projects heads h+2 ----
            for h in range(H):
                pt_t = [pts.tile([128, S], BF16, tag="pt", name=f"pt{h}_{sk}")
                        for sk in range(NT_S)]
                # weight slice j covers heads 2j/2j+1, first needed in iter
                # 2j-2; load at iter 2j-3 (slot j-2 frees at iter 2j-5).
                if h % 2 == 1 and (h + 3) // 2 < H // 2:
                    load_wqk2((h + 3) // 2)
                # wv group g first needed in iter 4g-3 (wv0/wv1 in prologue).
                if h == 2:
                    load_wv(2)
                elif h == 6:
                    load_wv(3)

                # vproj for group g spread over iters 4g-4..4g-1, two
                # s-tiles per iter (g0 was done in the prologue).
                vg, vst0, vnst = None, 0, 0
                if h <= 11:
                    vg, vst0, vnst = h // 4 + 1, (h % 4) * 2, 2
                    if vst0 == 0:
                        v4_alloc(vg)

                if h < H - 2:
                    scores_pair(h, pt_t, 0)
                    qproj(h + 2)
                    scores_pair(h, pt_t, 2)
                    kproj(h + 2)
                    scores_pair(h, pt_t, 4)
                    if vnst:
                        vproj_part(vg, vst0, vnst)
                    scores_pair(h, pt_t, 6)
                else:
                    # iters 14/15 have no projection work to hide exp()
                    # under, so pull in out-projection accumulation for
                    # chunks st=4,5 (e2c=0) using the idle pa psum tiles
                    # and separately-streamed Wo tiles (et 0..13).
                    if h == H - 2:
                        opull = [paps.tile([128, 512], F32, tag="pa",
                                           name=f"opull_{i}")
                                 for i in range(2)]
                        _CARRY["opull"] = opull
                    opull = _CARRY["opull"]

                    def opull_ets(e0, e1):
                        for et in range(e0, e1):
                            wo2_t = wo2p.tile([128, 512], BF16, tag="wo2",
                                              name=f"wo2_{et}")
                            nc.scalar.dma_start(
                                out=wo2_t,
                                in_=woT.ap()[et * 128:(et + 1) * 128, 0:512])
                            for i in range(2):
                                nc.tensor.matmul(
                                    opull[i],
                                    aoT_sb[:, et, (4 + i) * 128:(5 + i) * 128],
                                    wo2_t,
                                    start=(et == 0),
                                    stop=False,
                                )

                    base = 0 if h == H - 2 else 7
                    scores_pair(h, pt_t, 0)
                    opull_ets(base, base + 2)
                    scores_pair(h, pt_t, 2)
                    opull_ets(base + 2, base + 4)
                    scores_pair(h, pt_t, 4)
                    opull_ets(base + 4, base + 6)
                    scores_pair(h, pt_t, 6)
                    opull_ets(base + 6, base + 7)
                pv(h, pt_t)

            # ---- output projection: out[s, e2] = aoT.T @ WoT ----
            # 4 column passes of 8 chunks; Wo streamed per (e-tile, pass).
            for e2c in range(4):
                chunks = []
                for i in range(2):
                    t = ssps.tile([128, S], F32, tag="ss", name=f"oss_{e2c}_{i}")
                    chunks.append(t[:, 0:512])
                    chunks.append(t[:, 512:1024])
                for i in range(2):
                    if e2c == 0:
                        chunks.append(_CARRY["opull"][i])
                    else:
                        chunks.append(paps.tile([128, 512], F32, tag="pa",
                                                name=f"opa_{e2c}_{i}"))
                for i in range(2):
                    chunks.append(opps.tile([128, 512], F32, tag="op",
                                            name=f"oop_{e2c}_{i}"))
                for et in range(NT_E):
                    if e2c == 0 and et < len(wo_pre):
                        wo_t = wo_pre[et]
                    else:
                        wo_t = wop.tile([128, 512], BF16, tag="wo",
                                        name=f"wo_{e2c}_{et}")
                        nc.gpsimd.dma_start(
                            out=wo_t,
                            in_=woT.ap()[et * 128:(et + 1) * 128,
                                         e2c * 512:(e2c + 1) * 512])
                    for st in range(NT_S):
                        if e2c == 0 and st in (4, 5) and et < 14:
                            continue  # accumulated during iters 14/15
                        nc.tensor.matmul(
                            chunks[st],
                            aoT_sb[:, et, st * 128:(st + 1) * 128],
                            wo_t,
                            start=(et == 0),
                            stop=(et == NT_E - 1),
                        )
                        if et == NT_E - 1:
                            ot = outsb.tile([128, 512], F32, tag="outt",
                                            name=f"ot_{e2c}_{st}")
                            if st % 2 == 0:
                                nc.vector.tensor_copy(out=ot, in_=chunks[st])
                            else:
                                nc.scalar.copy(out=ot, in_=chunks[st])
                            nc.sync.dma_start(
                                out=out.ap()[st * 128:(st + 1) * 128,
                                             e2c * 512:(e2c + 1) * 512],
                                in_=ot,
                            )

    nc.compile()
    return nc


def _get_nc():
    if "nc" not in _CACHED:
        _CACHED["nc"] = _build()
    return _CACHED["nc"]


def _numpy_reference(query, key, value, attention_mask,
                     Wq, bq, Wk, bk, Wv, bv, Wo, bo):
    # general fallback (only used when attention_mask isn't all ones)
    Bb, SQ, _ = query.shape
    SK = key.shape[1]
    q = query @ Wq.T + bq
    k = key @ Wk.T + bk
    v = value @ Wv.T + bv
    q = q.reshape(Bb, SQ, H, HD).transpose(0, 2, 1, 3)
    k = k.reshape(Bb, SK, H, HD).transpose(0, 2, 1, 3)
    v = v.reshape(Bb, SK, H, HD).transpose(0, 2, 1, 3)
    scores = np.einsum("bhqd,bhkd->bhqk", q, k) * (HD ** -0.5)
    scores = np.where(attention_mask[:, None, :, :] == 0,
                      np.float32(-1e10), scores)
    scores -= scores.max(-1, keepdims=True)
    p = np.exp(scores)
    p /= p.sum(-1, keepdims=True)
    o = np.einsum("bhqk,bhkd->bhqd", p, v)
    o = o.transpose(0, 2, 1, 3).reshape(Bb, SQ, E)
    return (o @ Wo.T + bo).astype(np.float32)


def _prepare_in_maps(inputs):
    query = np.asarray(inputs["query"], dtype=np.float32)
    key = np.asarray(inputs["key"], dtype=np.float32)
    value = np.asarray(inputs["value"], dtype=np.float32)
    Wq = np.asarray(inputs["Wq"], dtype=np.float32)
    bq = np.asarray(inputs["bq"], dtype=np.float32)
    Wk = np.asarray(inputs["Wk"], dtype=np.float32)
    bk = np.asarray(inputs["bk"], dtype=np.float32)
    Wv = np.asarray(inputs["Wv"], dtype=np.float32)
    Wo = np.asarray(inputs["Wo"], dtype=np.float32)

    scale = np.float32(HD ** -0.5)
    bf = ml_dtypes.bfloat16
    wqT = np.ascontiguousarray((Wq.T * scale).astype(bf))
    wkT = np.ascontiguousarray(Wk.T.astype(bf))
    wvT = np.ascontiguousarray(Wv.T.astype(bf))
    woT = np.ascontiguousarray(Wo.T.astype(bf))
    bq_s = (bq * scale).astype(np.float32)

    in_maps = []
    for b in range(B):
        in_maps.append({
            "xqT": np.ascontiguousarray(query[b].T.astype(bf)),
            "xkT": np.ascontiguousarray(key[b].T.astype(bf)),
            "xvT": np.ascontiguousarray(value[b].T.astype(bf)),
            "wqT": wqT, "wkT": wkT, "wvT": wvT, "woT": woT,
            "bq": bq_s, "bk": bk.astype(np.float32),
        })
    return in_maps


def run_on_device(inputs, **spmd_kwargs):
    """Run the bass kernel; returns (out [B,S,E] f32, BassKernelResults)."""
    in_maps = _prepare_in_maps(inputs)
    Wo = np.asarray(inputs["Wo"], dtype=np.float64)
    bv = np.asarray(inputs["bv"], dtype=np.float64)
    bo = np.asarray(inputs["bo"], dtype=np.float64)
    bo_eff = (Wo @ bv + bo).astype(np.float32)
    res = run_bass_kernel_spmd(_get_nc(), in_maps,
                               core_ids=list(range(B)), **spmd_kwargs)
    out = np.stack([res.results[b]["out"] for b in range(B)], axis=0)
    return (out + bo_eff).astype(np.float32), res


def kernel(**inputs):
    mask = np.asarray(inputs["attention_mask"])
    if not mask.all():
        return _numpy_reference(
            np.asarray(inputs["query"], dtype=np.float32),
            np.asarray(inputs["key"], dtype=np.float32),
            np.asarray(inputs["value"], dtype=np.float32), mask,
            np.asarray(inputs["Wq"], dtype=np.float32),
            np.asarray(inputs["bq"], dtype=np.float32),
            np.asarray(inputs["Wk"], dtype=np.float32),
            np.asarray(inputs["bk"], dtype=np.float32),
            np.asarray(inputs["Wv"], dtype=np.float32),
            np.asarray(inputs["bv"], dtype=np.float32),
            np.asarray(inputs["Wo"], dtype=np.float32),
            np.asarray(inputs["bo"], dtype=np.float32))
    out, _ = run_on_device(inputs)
    return out


# revision 32
# speedup vs baseline: 1.3752x; 1.0200x over previous
"""CrossAttention kernel for 8 Trainium2 NeuronCores.

Problem (hardcoded): B=8, SQ=SK=1024, Q_DIM=2048, KV_DIM=1024, E_DIM=2048,
H=16 heads, HD=128.  out = softmax((X_q Wq^T + bq)(X_k Wk^T + bk)^T / sqrt(HD))
                            @ (X_v Wv^T + bv) @ Wo^T + bo

Sharding: data-parallel over batch — each of the 8 cores computes one batch
element end-to-end; no collectives.

Per-core dataflow (all matmuls bf16, f32 PSUM accumulation), software-
pipelined per head so the scalar engine's exp() hides under the next head's
projections:

  iter h: [scores(h) tiles interleaved with qproj(h+1)/kproj(h+1)/vproj part]
          then PV(h) -> ao_h -> DMA-transpose into aoT[:, h, :].
  - qT/kT produced in [e, s] layout (weight stationary).
  - v produced directly in [s, e] layout (xvT stationary, wv moving) with a
    ones column per head block => softmax denominators ride along as PV
    output column 128.  bv is folded into bo on the host (softmax rows sum
    to 1, so + bv passes through attention exactly).
  - out = aoT.T @ WoT accumulated over e-tiles with Wo streamed from DRAM;
    chunks copied+stored as they complete.
"""

import sys

sys.path.insert(0, "/opt/trn_rl_repo")

import numpy as np
import ml_dtypes

import concourse.tile as tile
from concourse import bacc
import concourse.mybir as mybir
from concourse.bass_utils import run_bass_kernel_spmd

F32 = mybir.dt.float32
BF16 = mybir.dt.bfloat16
ACT_IDENT = mybir.ActivationFunctionType.Identity
ACT_COPY = mybir.ActivationFunctionType.Copy
ACT_EXP = mybir.ActivationFunctionType.Exp

B = 8
S = 1024          # SQ == SK
DQ = 2048         # query input dim
DKV = 1024        # key/value input dim
E = 2048          # embed dim
H = 16            # heads
HD = 128          # head dim
NT_S = S // 128   # 8 seq tiles
NT_E = E // 128   # 16 e tiles (== heads)
NT_DQ = DQ // 128
NT_DKV = DKV // 128
VROW = HD + 1     # head block in v group incl. ones column

_CACHED = {}


def _build():
    nc = bacc.Bacc("TRN2", target_bir_lowering=False, debug=False)

    xqT = nc.dram_tensor("xqT", [DQ, S], BF16, kind="ExternalInput")
    xkT = nc.dram_tensor("xkT", [DKV, S], BF16, kind="ExternalInput")
    xvT = nc.dram_tensor("xvT", [DKV, S], BF16, kind="ExternalInput")
    wqT = nc.dram_tensor("wqT", [DQ, E], BF16, kind="ExternalInput")
    wkT = nc.dram_tensor("wkT", [DKV, E], BF16, kind="ExternalInput")
    wvT = nc.dram_tensor("wvT", [DKV, E], BF16, kind="ExternalInput")
    woT = nc.dram_tensor("woT", [E, E], BF16, kind="ExternalInput")
    bq = nc.dram_tensor("bq", [E], F32, kind="ExternalInput")
    bk = nc.dram_tensor("bk", [E], F32, kind="ExternalInput")
    out = nc.dram_tensor("out", [S, E], F32, kind="ExternalOutput")

    xqT_r = xqT.rearrange("(t p) s -> p t s", p=128)
    xkT_r = xkT.rearrange("(t p) s -> p t s", p=128)
    xvT_r = xvT.rearrange("(t p) s -> p t s", p=128)
    wqT_r = wqT.rearrange("(t p) e -> p t e", p=128)
    wkT_r = wkT.rearrange("(t p) e -> p t e", p=128)
    wvT_r = wvT.rearrange("(t p) e -> p t e", p=128)

    with tile.TileContext(nc) as tc:
        with (
            tc.tile_pool(name="persist", bufs=1) as persist,
            tc.tile_pool(name="qk", bufs=3) as qkp,
            tc.tile_pool(name="v4p", bufs=2) as v4p,
            tc.tile_pool(name="pts", bufs=9) as pts,
            tc.tile_pool(name="aohp", bufs=2) as aohp,
            tc.tile_pool(name="wqp", bufs=2) as wqp,
            tc.tile_pool(name="wkp", bufs=2) as wkp,
            tc.tile_pool(name="wvp", bufs=2) as wvp,
            tc.tile_pool(name="wop", bufs=10) as wop,
            tc.tile_pool(name="outsb", bufs=3) as outsb,
            tc.tile_pool(name="small", bufs=4) as small,
            tc.tile_pool(name="ssps", bufs=2, space="PSUM") as ssps,
            tc.tile_pool(name="paps", bufs=2, space="PSUM") as paps,
            tc.tile_pool(name="opps", bufs=2, space="PSUM") as opps,
        ):
            # ---- resident inputs / constants ----
            bq_sb = persist.tile([128, NT_E], F32, tag="bq")
            bk_sb = persist.tile([128, NT_E], F32, tag="bk")
            nc.gpsimd.dma_start(out=bq_sb, in_=bq.rearrange("(t p) -> p t", p=128))
            nc.gpsimd.dma_start(out=bk_sb, in_=bk.rearrange("(t p) -> p t", p=128))

            xq_sb = persist.tile([128, NT_DQ, S], BF16, tag="xq")
            xk_sb = persist.tile([128, NT_DKV, S], BF16, tag="xk")
            xv_sb = persist.tile([128, NT_DKV, S], BF16, tag="xv")
            aoT_sb = persist.tile([128, NT_E, S], BF16, tag="aoT")

            # weight slices (wq/wk cover 2 heads per slice)
            wq_sl = {}
            wk_sl = {}
            wv_sl = {}

            def load_wqk2(j, eng=None):  # heads 2j, 2j+1
                eng = eng or nc.scalar
                wq_sl[j] = wqp.tile([128, NT_DQ, 256], BF16, tag="wq",
                                    name=f"wq2_{j}")
                eng.dma_start(
                    out=wq_sl[j], in_=wqT_r[:, :, 2 * j * 128:(2 * j + 2) * 128])
                wk_sl[j] = wkp.tile([128, NT_DKV, 256], BF16, tag="wk",
                                    name=f"wk2_{j}")
                eng.dma_start(
                    out=wk_sl[j], in_=wkT_r[:, :, 2 * j * 128:(2 * j + 2) * 128])

            def load_wv(g, eng=None):  # heads 4g..4g+3
                eng = eng or nc.scalar
                wv_sl[g] = wvp.tile([128, NT_DKV, 512], BF16, tag="wv",
                                    name=f"wv_{g}")
                eng.dma_start(
                    out=wv_sl[g], in_=wvT_r[:, :, 4 * g * 128:(4 * g + 4) * 128])

            # Prologue DMAs: ALL on the sync queue, in exact consumption
            # order (the DMA device serves transfers in arrival order, and
            # cross-queue ordering is uncontrolled): kproj needs wk+xk,
            # then vproj g0 needs wv+xv, then qproj needs wq+xq.
            wk_sl[0] = wkp.tile([128, NT_DKV, 256], BF16, tag="wk", name="wk2_0")
            nc.sync.dma_start(out=wk_sl[0][:, 0:4, :], in_=wkT_r[:, 0:4, 0:256])
            nc.sync.dma_start(out=wk_sl[0][:, 4:8, :], in_=wkT_r[:, 4:8, 0:256])
            for d in range(NT_DKV):
                nc.sync.dma_start(out=xk_sb[:, d, :], in_=xkT_r[:, d, :])
            load_wv(0, eng=nc.sync)
            for d in range(NT_DKV):
                nc.sync.dma_start(out=xv_sb[:, d, :], in_=xvT_r[:, d, :])
            wq_sl[0] = wqp.tile([128, NT_DQ, 256], BF16, tag="wq", name="wq2_0")
            nc.sync.dma_start(out=wq_sl[0], in_=wqT_r[:, :, 0:256])
            for d in range(NT_DQ):
                nc.sync.dma_start(out=xq_sb[:, d, :], in_=xqT_r[:, d, :])
            load_wqk2(1, eng=nc.sync)
            load_wv(1, eng=nc.sync)
            # prefetch first out-projection weight tiles (parked until the
            # epilogue; also throttles the gpsimd wo stream via pool slots)
            wo_pre = []
            for et in range(6):
                wo_t = wop.tile([128, 512], BF16, tag="wo", name=f"wo_0_{et}")
                nc.sync.dma_start(
                    out=wo_t, in_=woT.ap()[et * 128:(et + 1) * 128, 0:512])
                wo_pre.append(wo_t)

            qT = {}
            kT = {}
            v4 = {}
            _CARRY = {}

            def qproj(h):
                """qT[h] [e128, s] <- sum_d wq-block.T @ xq (2 psum chunks)."""
                sl = wq_sl[h // 2]
                hoff = (h % 2) * 128
                qT[h] = qkp.tile([128, S], BF16, tag="qT", name=f"qT_{h}")
                for c in range(2):
                    ps = paps.tile([128, 512], F32, tag="pa", name=f"qps_{h}_{c}")
                    for d in range(NT_DQ):
                        nc.tensor.matmul(
                            ps,
                            sl[:, d, hoff:hoff + 128],
                            xq_sb[:, d, c * 512:(c + 1) * 512],
                            start=(d == 0),
                            stop=(d == NT_DQ - 1),
                        )
                    nc.scalar.activation(
                        out=qT[h][:, c * 512:(c + 1) * 512], in_=ps,
                        func=ACT_IDENT, bias=bq_sb[:, h:h + 1], scale=1.0)

            def kproj(h):
                sl = wk_sl[h // 2]
                hoff = (h % 2) * 128
                kT[h] = qkp.tile([128, S], BF16, tag="kT", name=f"kT_{h}")
                for c in range(2):
                    ps = paps.tile([128, 512], F32, tag="pa", name=f"kps_{h}_{c}")
                    for d in range(NT_DKV):
                        nc.tensor.matmul(
                            ps,
                            sl[:, d, hoff:hoff + 128],
                            xk_sb[:, d, c * 512:(c + 1) * 512],
                            start=(d == 0),
                            stop=(d == NT_DKV - 1),
                        )
                    nc.scalar.activation(
                        out=kT[h][:, c * 512:(c + 1) * 512], in_=ps,
                        func=ACT_IDENT, bias=bk_sb[:, h:h + 1], scale=1.0)

            def v4_alloc(g):
                v4[g] = v4p.tile([128, NT_S, 4 * VROW], BF16, tag="v4",
                                 name=f"v4_{g}")
                nc.vector.memset(
                    v4[g].rearrange("p t (h c) -> p t h c", c=VROW)
                    [:, :, :, HD:], 1.0)

            def vproj_part(g, st0, nst):
                """v4[g][:, st, :] <- (xv st-block).T @ wv-group, s-tiles
                st0..st0+nst-1; heads 4g..4g+3 with ones columns."""
                sl = wv_sl[g]
                for st in range(st0, st0 + nst):
                    ps = paps.tile([128, 512], F32, tag="pa",
                                   name=f"vps_{g}_{st}")
                    for d in range(NT_DKV):
                        nc.tensor.matmul(
                            ps,
                            xv_sb[:, d, st * 128:(st + 1) * 128],
                            sl[:, d, :],
                            start=(d == 0),
                            stop=(d == NT_DKV - 1),
                        )
                    nc.vector.tensor_copy(
                        out=v4[g][:, st, :].rearrange(
                            "p (h c) -> p h c", c=VROW)[:, :, 0:HD],
                        in_=ps.rearrange("p (h c) -> p h c", c=HD),
                    )

            def scores_pair(h, pt_t, s0):
                """two score tiles s0, s0+1: scoresT[sk,sq] -> exp -> pt."""
                for sk in (s0, s0 + 1):
                    ss = ssps.tile([128, S], F32, tag="ss",
                                   name=f"ss_{h}_{sk}")
                    for c in range(2):
                        nc.tensor.matmul(
                            ss[:, c * 512:(c + 1) * 512],
                            kT[h][:, sk * 128:(sk + 1) * 128],
                            qT[h][:, c * 512:(c + 1) * 512],
                            start=True,
                            stop=True,
                        )
                    nc.scalar.activation(
                        out=pt_t[sk], in_=ss, func=ACT_EXP, bias=0.0, scale=1.0)

            def pv(h, pt_t):
                g, hig = h // 4, h % 4
                ao_h = aohp.tile([128, NT_S, HD], BF16, tag="aoh",
                                 name=f"aoh_{h}")
                for st in range(NT_S):
                    op = opps.tile([128, 512], F32, tag="op",
                                   name=f"op_{h}_{st}")
                    for sk in range(NT_S):
                        nc.tensor.matmul(
                            op[:, 0:VROW],
                            pt_t[sk][:, st * 128:(st + 1) * 128],
                            v4[g][:, sk, hig * VROW:(hig + 1) * VROW],
                            start=(sk == 0),
                            stop=(sk == NT_S - 1),
                        )
                    rec = small.tile([128, 1], F32, tag="rec",
                                     name=f"rec_{h}_{st}")
                    nc.vector.reciprocal(out=rec, in_=op[:, HD:VROW])
                    nc.scalar.activation(
                        out=ao_h[:, st, :], in_=op[:, 0:HD],
                        func=ACT_COPY, bias=0.0, scale=rec)
                nc.sync.dma_start_transpose(
                    out=aoT_sb[:, h, :].rearrange("p (t c) -> p t c", c=128),
                    in_=ao_h)

            # ---- prologue compute, d-outer interleaved so PE consumption
            # rate (4 mms per d-tile) stays behind the DMA arrival rate ----
            def prologue_qk2(proj_wsl, proj_x, nt_d, bias_sb, dst, nm):
                dst[0] = qkp.tile([128, S], BF16, tag=nm, name=f"{nm}_0")
                dst[1] = qkp.tile([128, S], BF16, tag=nm, name=f"{nm}_1")
                ps0 = [paps.tile([128, 512], F32, tag="pa",
                                 name=f"{nm}p0_{c}") for c in range(2)]
                sst = ssps.tile([128, S], F32, tag="ss", name=f"{nm}p1")
                ps1 = [sst[:, 0:512], sst[:, 512:1024]]
                for d in range(nt_d):
                    for hh, pss in ((0, ps0), (1, ps1)):
                        for c in range(2):
                            nc.tensor.matmul(
                                pss[c],
                                proj_wsl[:, d, hh * 128:(hh + 1) * 128],
                                proj_x[:, d, c * 512:(c + 1) * 512],
                                start=(d == 0),
                                stop=(d == nt_d - 1),
                            )
                for hh, pss in ((0, ps0), (1, ps1)):
                    for c in range(2):
                        nc.scalar.activation(
                            out=dst[hh][:, c * 512:(c + 1) * 512], in_=pss[c],
                            func=ACT_IDENT, bias=bias_sb[:, hh:hh + 1],
                            scale=1.0)

            prologue_qk2(wk_sl[0], xk_sb, NT_DKV, bk_sb, kT, "kT")
            v4_alloc(0)
            # vproj g0: two rounds of 4 s-tiles, d-outer (2 pa + 1 ss tile)
            for rnd in range(2):
                pv_ps = [paps.tile([128, 512], F32, tag="pa",
                                   name=f"vp{rnd}_{i}") for i in range(2)]
                sst = ssps.tile([128, S], F32, tag="ss", name=f"vp{rnd}ss")
                pv_ps.append(sst[:, 0:512])
                pv_ps.append(sst[:, 512:1024])
                for d in range(NT_DKV):
                    for i in range(4):
                        st = rnd * 4 + i
                        nc.tensor.matmul(
                            pv_ps[i],
                            xv_sb[:, d, st * 128:(st + 1) * 128],
                            wv_sl[0][:, d, :],
                            start=(d == 0),
                            stop=(d == NT_DKV - 1),
                        )
                for i in range(4):
                    st = rnd * 4 + i
                    nc.vector.tensor_copy(
                        out=v4[0][:, st, :].rearrange(
                            "p (h c) -> p h c", c=VROW)[:, :, 0:HD],
                        in_=pv_ps[i].rearrange("p (h c) -> p h c", c=HD),
                    )
            prologue_qk2(wq_sl[0], xq_sb, NT_DQ, bq_sb, qT, "qT")

            # ---- main loop over heads; iter h projects heads h+2 ----
            for h in range(H):
                pt_t = [pts.tile([128, S], BF16, tag="pt", name=f"pt{h}_{sk}")
                        for sk in range(NT_S)]
                # weight slice j covers heads 2j/2j+1, first needed in iter
                # 2j-2; load at iter 2j-3 (slot j-2 frees at iter 2j-5).
                if h % 2 == 1 and (h + 3) // 2 < H // 2:
                    load_wqk2((h + 3) // 2)
                # wv group g first needed in iter 4g-3 (wv0/wv1 in prologue).
                if h == 2:
                    load_wv(2)
                elif h == 6:
                    load_wv(3)

                # vproj for group g spread over iters 4g-4..4g-1, two
                # s-tiles per iter (g0 was done in the prologue).
                vg, vst0, vnst = None, 0, 0
                if h <= 11:
                    vg, vst0, vnst = h // 4 + 1, (h % 4) * 2, 2
                    if vst0 == 0:
                        v4_alloc(vg)

                if h < H - 2:
                    scores_pair(h, pt_t, 0)
                    qproj(h + 2)
                    scores_pair(h, pt_t, 2)
                    kproj(h + 2)
                    scores_pair(h, pt_t, 4)
                    if vnst:
                        vproj_part(vg, vst0, vnst)
                    scores_pair(h, pt_t, 6)
                else:
                    # iters 14/15 have no projection work to hide exp()
                    # under, so pull in out-projection accumulation for
                    # chunks st=4,5 (e2c=0) using the idle pa psum tiles
                    # and separately-streamed Wo tiles (et 0..13).
                    if h == H - 2:
                        opull = [paps.tile([128, 512], F32, tag="pa",
                                           name=f"opull_{i}")
                                 for i in range(2)]
                        _CARRY["opull"] = opull
                    opull = _CARRY["opull"]

                    def opull_ets(e0, e1):
                        # uses the wo_pre tiles parked since the prologue
                        for et in range(e0, e1):
                            for i in range(2):
                                nc.tensor.matmul(
                                    opull[i],
                                    aoT_sb[:, et, (4 + i) * 128:(5 + i) * 128],
                                    wo_pre[et],
                                    start=(et == 0),
                                    stop=False,
                                )

                    base = 0 if h == H - 2 else 3
                    scores_pair(h, pt_t, 0)
                    opull_ets(base, base + 1)
                    scores_pair(h, pt_t, 2)
                    opull_ets(base + 1, base + 2)
                    scores_pair(h, pt_t, 4)
                    opull_ets(base + 2, base + 3)
                    scores_pair(h, pt_t, 6)
                pv(h, pt_t)

            # ---- output projection: out[s, e2] = aoT.T @ WoT ----
            # 4 column passes of 8 chunks; Wo streamed per (e-tile, pass).
            for e2c in range(4):
                chunks = []
                for i in range(2):
                    t = ssps.tile([128, S], F32, tag="ss", name=f"oss_{e2c}_{i}")
                    chunks.append(t[:, 0:512])
                    chunks.append(t[:, 512:1024])
                for i in range(2):
                    if e2c == 0:
                        chunks.append(_CARRY["opull"][i])
                    else:
                        chunks.append(paps.tile([128, 512], F32, tag="pa",
                                                name=f"opa_{e2c}_{i}"))
                for i in range(2):
                    chunks.append(opps.tile([128, 512], F32, tag="op",
                                            name=f"oop_{e2c}_{i}"))
                def get_wo(et):
                    if e2c == 0 and et < len(wo_pre):
                        return wo_pre[et]
                    wo_t = wop.tile([128, 512], BF16, tag="wo",
                                    name=f"wo_{e2c}_{et}")
                    nc.gpsimd.dma_start(
                        out=wo_t,
                        in_=woT.ap()[et * 128:(et + 1) * 128,
                                     e2c * 512:(e2c + 1) * 512])
                    return wo_t

                # et-major streaming phase (et 0..7)
                for et in range(8):
                    wo_t = get_wo(et)
                    for st in range(NT_S):
                        if e2c == 0 and st in (4, 5) and et < 6:
                            continue  # accumulated during iters 14/15
                        nc.tensor.matmul(
                            chunks[st],
                            aoT_sb[:, et, st * 128:(st + 1) * 128],
                            wo_t,
                            start=(et == 0),
                            stop=False,
                        )
                # staggered tail: each chunk finishes its last 8 ets, then
                # copy+store immediately so completions pipeline out at a
                # spacing (~1.7us) above the copy+DMA drain rate
                wo_tail = {et: get_wo(et) for et in range(8, NT_E)}
                for st in range(NT_S):
                    for et in range(8, NT_E):
                        nc.tensor.matmul(
                            chunks[st],
                            aoT_sb[:, et, st * 128:(st + 1) * 128],
                            wo_tail[et],
                            start=False,
                            stop=(et == NT_E - 1),
                        )
                    ot = outsb.tile([128, 512], F32, tag="outt",
                                    name=f"ot_{e2c}_{st}")
                    if st % 2 == 0:
                        nc.vector.tensor_copy(out=ot, in_=chunks[st])
                        eng = nc.sync
                    else:
                        nc.scalar.copy(out=ot, in_=chunks[st])
                        eng = nc.scalar
                    eng.dma_start(
                        out=out.ap()[st * 128:(st + 1) * 128,
                                     e2c * 512:(e2c + 1) * 512],
                        in_=ot,
                    )

    nc.compile()
    return nc


def _get_nc():
    if "nc" not in _CACHED:
        _CACHED["nc"] = _build()
    return _CACHED["nc"]


def _numpy_reference(query, key, value, attention_mask,
                     Wq, bq, Wk, bk, Wv, bv, Wo, bo):
    # general fallback (only used when attention_mask isn't all ones)
    Bb, SQ, _ = query.shape
    SK = key.shape[1]
    q = query @ Wq.T + bq
    k = key @ Wk.T + bk
    v = value @ Wv.T + bv
    q = q.reshape(Bb, SQ, H, HD).transpose(0, 2, 1, 3)
    k = k.reshape(Bb, SK, H, HD).transpose(0, 2, 1, 3)
    v = v.reshape(Bb, SK, H, HD).transpose(0, 2, 1, 3)
    scores = np.einsum("bhqd,bhkd->bhqk", q, k) * (HD ** -0.5)
    scores = np.where(attention_mask[:, None, :, :] == 0,
                      np.float32(-1e10), scores)
    scores -= scores.max(-1, keepdims=True)
    p = np.exp(scores)
    p /= p.sum(-1, keepdims=True)
    o = np.einsum("bhqk,bhkd->bhqd", p, v)
    o = o.transpose(0, 2, 1, 3).reshape(Bb, SQ, E)
    return (o @ Wo.T + bo).astype(np.float32)


def _prepare_in_maps(inputs):
    query = np.asarray(inputs["query"], dtype=np.float32)
    key = np.asarray(inputs["key"], dtype=np.float32)
    value = np.asarray(inputs["value"], dtype=np.float32)
    Wq = np.asarray(inputs["Wq"], dtype=np.float32)
    bq = np.asarray(inputs["bq"], dtype=np.float32)
    Wk = np.asarray(inputs["Wk"], dtype=np.float32)
    bk = np.asarray(inputs["bk"], dtype=np.float32)
    Wv = np.asarray(inputs["Wv"], dtype=np.float32)
    Wo = np.asarray(inputs["Wo"], dtype=np.float32)

    scale = np.float32(HD ** -0.5)
    bf = ml_dtypes.bfloat16
    wqT = np.ascontiguousarray((Wq.T * scale).astype(bf))
    wkT = np.ascontiguousarray(Wk.T.astype(bf))
    wvT = np.ascontiguousarray(Wv.T.astype(bf))
    woT = np.ascontiguousarray(Wo.T.astype(bf))
    bq_s = (bq * scale).astype(np.float32)

    in_maps = []
    for b in range(B):
        in_maps.append({
            "xqT": np.ascontiguousarray(query[b].T.astype(bf)),
            "xkT": np.ascontiguousarray(key[b].T.astype(bf)),
            "xvT": np.ascontiguousarray(value[b].T.astype(bf)),
            "wqT": wqT, "wkT": wkT, "wvT": wvT, "woT": woT,
            "bq": bq_s, "bk": bk.astype(np.float32),
        })
    return in_maps


def run_on_device(inputs, **spmd_kwargs):
    """Run the bass kernel; returns (out [B,S,E] f32, BassKernelResults)."""
    in_maps = _prepare_in_maps(inputs)
    Wo = np.asarray(inputs["Wo"], dtype=np.float64)
    bv = np.asarray(inputs["bv"], dtype=np.float64)
    bo = np.asarray(inputs["bo"], dtype=np.float64)
    bo_eff = (Wo @ bv + bo).astype(np.float32)
    res = run_bass_kernel_spmd(_get_nc(), in_maps,
                               core_ids=list(range(B)), **spmd_kwargs)
    out = np.stack([res.results[b]["out"] for b in range(B)], axis=0)
    return (out + bo_eff).astype(np.float32), res


def kernel(**inputs):
    mask = np.asarray(inputs["attention_mask"])
    if not mask.all():
        return _numpy_reference(
            np.asarray(inputs["query"], dtype=np.float32),
            np.asarray(inputs["key"], dtype=np.float32),
            np.asarray(inputs["value"], dtype=np.float32), mask,
            np.asarray(inputs["Wq"], dtype=np.float32),
            np.asarray(inputs["bq"], dtype=np.float32),
            np.asarray(inputs["Wk"], dtype=np.float32),
            np.asarray(inputs["bk"], dtype=np.float32),
            np.asarray(inputs["Wv"], dtype=np.float32),
            np.asarray(inputs["bv"], dtype=np.float32),
            np.asarray(inputs["Wo"], dtype=np.float32),
            np.asarray(inputs["bo"], dtype=np.float32))
    out, _ = run_on_device(inputs)
    return out


# revision 37
# speedup vs baseline: 1.4015x; 1.0191x over previous
"""CrossAttention kernel for 8 Trainium2 NeuronCores.

Problem (hardcoded): B=8, SQ=SK=1024, Q_DIM=2048, KV_DIM=1024, E_DIM=2048,
H=16 heads, HD=128.  out = softmax((X_q Wq^T + bq)(X_k Wk^T + bk)^T / sqrt(HD))
                            @ (X_v Wv^T + bv) @ Wo^T + bo

Sharding: data-parallel over batch — each of the 8 cores computes one batch
element end-to-end; no collectives.

Per-core dataflow (all matmuls bf16, f32 PSUM accumulation), software-
pipelined per head so the scalar engine's exp() hides under the next head's
projections:

  iter h: [scores(h) tiles interleaved with qproj(h+1)/kproj(h+1)/vproj part]
          then PV(h) -> ao_h -> DMA-transpose into aoT[:, h, :].
  - qT/kT produced in [e, s] layout (weight stationary).
  - v produced directly in [s, e] layout (xvT stationary, wv moving) with a
    ones column per head block => softmax denominators ride along as PV
    output column 128.  bv is folded into bo on the host (softmax rows sum
    to 1, so + bv passes through attention exactly).
  - out = aoT.T @ WoT accumulated over e-tiles with Wo streamed from DRAM;
    chunks copied+stored as they complete.
"""

import sys

sys.path.insert(0, "/opt/trn_rl_repo")

import numpy as np
import ml_dtypes

import concourse.tile as tile
from concourse import bacc
import concourse.mybir as mybir
from concourse.bass_utils import run_bass_kernel_spmd

F32 = mybir.dt.float32
BF16 = mybir.dt.bfloat16
ACT_IDENT = mybir.ActivationFunctionType.Identity
ACT_COPY = mybir.ActivationFunctionType.Copy
ACT_EXP = mybir.ActivationFunctionType.Exp

B = 8
S = 1024          # SQ == SK
DQ = 2048         # query input dim
DKV = 1024        # key/value input dim
E = 2048          # embed dim
H = 16            # heads
HD = 128          # head dim
NT_S = S // 128   # 8 seq tiles
NT_E = E // 128   # 16 e tiles (== heads)
NT_DQ = DQ // 128
NT_DKV = DKV // 128
VROW = HD + 1     # head block in v group incl. ones column

_CACHED = {}


def _build():
    nc = bacc.Bacc("TRN2", target_bir_lowering=False, debug=False)

    xqT = nc.dram_tensor("xqT", [DQ, S], BF16, kind="ExternalInput")
    xkT = nc.dram_tensor("xkT", [DKV, S], BF16, kind="ExternalInput")
    xvT = nc.dram_tensor("xvT", [DKV, S], BF16, kind="ExternalInput")
    wqT = nc.dram_tensor("wqT", [DQ, E], BF16, kind="ExternalInput")
    wkT = nc.dram_tensor("wkT", [DKV, E], BF16, kind="ExternalInput")
    wvT = nc.dram_tensor("wvT", [DKV, E], BF16, kind="ExternalInput")
    woT = nc.dram_tensor("woT", [E, E], BF16, kind="ExternalInput")
    bq = nc.dram_tensor("bq", [E], F32, kind="ExternalInput")
    bk = nc.dram_tensor("bk", [E], F32, kind="ExternalInput")
    out = nc.dram_tensor("out", [S, E], F32, kind="ExternalOutput")

    xqT_r = xqT.rearrange("(t p) s -> p t s", p=128)
    xkT_r = xkT.rearrange("(t p) s -> p t s", p=128)
    xvT_r = xvT.rearrange("(t p) s -> p t s", p=128)
    wqT_r = wqT.rearrange("(t p) e -> p t e", p=128)
    wkT_r = wkT.rearrange("(t p) e -> p t e", p=128)
    wvT_r = wvT.rearrange("(t p) e -> p t e", p=128)

    with tile.TileContext(nc) as tc:
        with (
            tc.tile_pool(name="persist", bufs=1) as persist,
            tc.tile_pool(name="qk", bufs=3) as qkp,
            tc.tile_pool(name="v4p", bufs=2) as v4p,
            tc.tile_pool(name="pts", bufs=9) as pts,
            tc.tile_pool(name="aohp", bufs=2) as aohp,
            tc.tile_pool(name="wqp", bufs=2) as wqp,
            tc.tile_pool(name="wkp", bufs=2) as wkp,
            tc.tile_pool(name="wvp", bufs=2) as wvp,
            tc.tile_pool(name="wop", bufs=10) as wop,
            tc.tile_pool(name="outsb", bufs=3) as outsb,
            tc.tile_pool(name="small", bufs=4) as small,
            tc.tile_pool(name="ssps", bufs=2, space="PSUM") as ssps,
            tc.tile_pool(name="paps", bufs=2, space="PSUM") as paps,
            tc.tile_pool(name="opps", bufs=2, space="PSUM") as opps,
        ):
            # ---- resident inputs / constants ----
            bq_sb = persist.tile([128, NT_E], F32, tag="bq")
            bk_sb = persist.tile([128, NT_E], F32, tag="bk")
            nc.gpsimd.dma_start(out=bq_sb, in_=bq.rearrange("(t p) -> p t", p=128))
            nc.gpsimd.dma_start(out=bk_sb, in_=bk.rearrange("(t p) -> p t", p=128))

            xq_sb = persist.tile([128, NT_DQ, S], BF16, tag="xq")
            xk_sb = persist.tile([128, NT_DKV, S], BF16, tag="xk")
            xv_sb = persist.tile([128, NT_DKV, S], BF16, tag="xv")
            aoT_sb = persist.tile([128, NT_E, S], BF16, tag="aoT")

            # weight slices (wq/wk cover 2 heads per slice)
            wq_sl = {}
            wk_sl = {}
            wv_sl = {}

            def load_wqk2(j, eng=None):  # heads 2j, 2j+1
                eng = eng or nc.scalar
                wq_sl[j] = wqp.tile([128, NT_DQ, 256], BF16, tag="wq",
                                    name=f"wq2_{j}")
                eng.dma_start(
                    out=wq_sl[j], in_=wqT_r[:, :, 2 * j * 128:(2 * j + 2) * 128])
                wk_sl[j] = wkp.tile([128, NT_DKV, 256], BF16, tag="wk",
                                    name=f"wk2_{j}")
                eng.dma_start(
                    out=wk_sl[j], in_=wkT_r[:, :, 2 * j * 128:(2 * j + 2) * 128])

            def load_wv(g, eng=None):  # heads 4g..4g+3
                eng = eng or nc.scalar
                wv_sl[g] = wvp.tile([128, NT_DKV, 512], BF16, tag="wv",
                                    name=f"wv_{g}")
                eng.dma_start(
                    out=wv_sl[g], in_=wvT_r[:, :, 4 * g * 128:(4 * g + 4) * 128])

            # Prologue DMAs: ALL on the sync queue, in exact consumption
            # order (the DMA device serves transfers in arrival order, and
            # cross-queue ordering is uncontrolled): kproj needs wk+xk,
            # then vproj g0 needs wv+xv, then qproj needs wq+xq.
            # first weight halves go on the scalar queue so their DGE
            # latency overlaps the sync-queue xk stream
            wk_sl[0] = wkp.tile([128, NT_DKV, 256], BF16, tag="wk", name="wk2_0")
            nc.scalar.dma_start(out=wk_sl[0][:, 0:4, :], in_=wkT_r[:, 0:4, 0:256])
            nc.scalar.dma_start(out=wk_sl[0][:, 4:8, :], in_=wkT_r[:, 4:8, 0:256])
            for d in range(NT_DKV):
                nc.sync.dma_start(out=xk_sb[:, d, :], in_=xkT_r[:, d, :])
            wv_sl[0] = wvp.tile([128, NT_DKV, 512], BF16, tag="wv", name="wv_0")
            nc.sync.dma_start(out=wv_sl[0][:, 0:4, :], in_=wvT_r[:, 0:4, 0:512])
            for d in range(4):
                nc.sync.dma_start(out=xv_sb[:, d, :], in_=xvT_r[:, d, :])
            nc.sync.dma_start(out=wv_sl[0][:, 4:8, :], in_=wvT_r[:, 4:8, 0:512])
            for d in range(4, NT_DKV):
                nc.sync.dma_start(out=xv_sb[:, d, :], in_=xvT_r[:, d, :])
            wq_sl[0] = wqp.tile([128, NT_DQ, 256], BF16, tag="wq", name="wq2_0")
            nc.sync.dma_start(out=wq_sl[0], in_=wqT_r[:, :, 0:256])
            for d in range(NT_DQ):
                nc.sync.dma_start(out=xq_sb[:, d, :], in_=xqT_r[:, d, :])
            load_wqk2(1, eng=nc.sync)
            load_wv(1, eng=nc.sync)
            # prefetch first out-projection weight tiles (parked until the
            # epilogue; also throttles the gpsimd wo stream via pool slots)
            wo_pre = []
            for et in range(8):
                wo_t = wop.tile([128, 512], BF16, tag="wo", name=f"wo_0_{et}")
                nc.sync.dma_start(
                    out=wo_t, in_=woT.ap()[et * 128:(et + 1) * 128, 0:512])
                wo_pre.append(wo_t)

            qT = {}
            kT = {}
            v4 = {}
            _CARRY = {}

            def qproj(h):
                """qT[h] [e128, s] <- sum_d wq-block.T @ xq (2 psum chunks)."""
                sl = wq_sl[h // 2]
                hoff = (h % 2) * 128
                qT[h] = qkp.tile([128, S], BF16, tag="qT", name=f"qT_{h}")
                for c in range(2):
                    ps = paps.tile([128, 512], F32, tag="pa", name=f"qps_{h}_{c}")
                    for d in range(NT_DQ):
                        nc.tensor.matmul(
                            ps,
                            sl[:, d, hoff:hoff + 128],
                            xq_sb[:, d, c * 512:(c + 1) * 512],
                            start=(d == 0),
                            stop=(d == NT_DQ - 1),
                        )
                    nc.scalar.activation(
                        out=qT[h][:, c * 512:(c + 1) * 512], in_=ps,
                        func=ACT_IDENT, bias=bq_sb[:, h:h + 1], scale=1.0)

            def kproj(h):
                sl = wk_sl[h // 2]
                hoff = (h % 2) * 128
                kT[h] = qkp.tile([128, S], BF16, tag="kT", name=f"kT_{h}")
                for c in range(2):
                    ps = paps.tile([128, 512], F32, tag="pa", name=f"kps_{h}_{c}")
                    for d in range(NT_DKV):
                        nc.tensor.matmul(
                            ps,
                            sl[:, d, hoff:hoff + 128],
                            xk_sb[:, d, c * 512:(c + 1) * 512],
                            start=(d == 0),
                            stop=(d == NT_DKV - 1),
                        )
                    nc.scalar.activation(
                        out=kT[h][:, c * 512:(c + 1) * 512], in_=ps,
                        func=ACT_IDENT, bias=bk_sb[:, h:h + 1], scale=1.0)

            def v4_alloc(g):
                v4[g] = v4p.tile([128, NT_S, 4 * VROW], BF16, tag="v4",
                                 name=f"v4_{g}")
                nc.vector.memset(
                    v4[g].rearrange("p t (h c) -> p t h c", c=VROW)
                    [:, :, :, HD:], 1.0)

            def vproj_part(g, st0, nst):
                """v4[g][:, st, :] <- (xv st-block).T @ wv-group, s-tiles
                st0..st0+nst-1; heads 4g..4g+3 with ones columns."""
                sl = wv_sl[g]
                for st in range(st0, st0 + nst):
                    ps = paps.tile([128, 512], F32, tag="pa",
                                   name=f"vps_{g}_{st}")
                    for d in range(NT_DKV):
                        nc.tensor.matmul(
                            ps,
                            xv_sb[:, d, st * 128:(st + 1) * 128],
                            sl[:, d, :],
                            start=(d == 0),
                            stop=(d == NT_DKV - 1),
                        )
                    nc.vector.tensor_copy(
                        out=v4[g][:, st, :].rearrange(
                            "p (h c) -> p h c", c=VROW)[:, :, 0:HD],
                        in_=ps.rearrange("p (h c) -> p h c", c=HD),
                    )

            def scores_pair(h, pt_t, s0):
                """two score tiles s0, s0+1: scoresT[sk,sq] -> exp -> pt."""
                for sk in (s0, s0 + 1):
                    ss = ssps.tile([128, S], F32, tag="ss",
                                   name=f"ss_{h}_{sk}")
                    for c in range(2):
                        nc.tensor.matmul(
                            ss[:, c * 512:(c + 1) * 512],
                            kT[h][:, sk * 128:(sk + 1) * 128],
                            qT[h][:, c * 512:(c + 1) * 512],
                            start=True,
                            stop=True,
                        )
                    nc.scalar.activation(
                        out=pt_t[sk], in_=ss, func=ACT_EXP, bias=0.0, scale=1.0)

            def pv(h, pt_t):
                g, hig = h // 4, h % 4
                ao_h = aohp.tile([128, NT_S, HD], BF16, tag="aoh",
                                 name=f"aoh_{h}")
                # 4 rotating accumulator slots: 2 op-pool tiles + both banks
                # of one ss tile (scores(h) has fully drained through exp by
                # now) — wide enough that the recip+scale drain never stalls
                # the PV matmul stream.
                ssa = ssps.tile([128, S], F32, tag="ss", name=f"pvss_{h}")
                opa = opps.tile([128, 512], F32, tag="op", name=f"opa_{h}")
                opb = opps.tile([128, 512], F32, tag="op", name=f"opb_{h}")
                chunk4 = [opa, opb, ssa[:, 0:512], ssa[:, 512:1024]]
                for st in range(NT_S):
                    op = chunk4[st % 4]
                    for sk in range(NT_S):
                        nc.tensor.matmul(
                            op[:, 0:VROW],
                            pt_t[sk][:, st * 128:(st + 1) * 128],
                            v4[g][:, sk, hig * VROW:(hig + 1) * VROW],
                            start=(sk == 0),
                            stop=(sk == NT_S - 1),
                        )
                    rec = small.tile([128, 1], F32, tag="rec",
                                     name=f"rec_{h}_{st}")
                    nc.vector.reciprocal(out=rec, in_=op[:, HD:VROW])
                    nc.vector.tensor_scalar_mul(
                        ao_h[:, st, :], op[:, 0:HD], rec)
                nc.sync.dma_start_transpose(
                    out=aoT_sb[:, h, :].rearrange("p (t c) -> p t c", c=128),
                    in_=ao_h)

            # ---- prologue compute, d-outer interleaved so PE consumption
            # rate (4 mms per d-tile) stays behind the DMA arrival rate ----
            def prologue_qk2(proj_wsl, proj_x, nt_d, bias_sb, dst, nm):
                dst[0] = qkp.tile([128, S], BF16, tag=nm, name=f"{nm}_0")
                dst[1] = qkp.tile([128, S], BF16, tag=nm, name=f"{nm}_1")
                ps0 = [paps.tile([128, 512], F32, tag="pa",
                                 name=f"{nm}p0_{c}") for c in range(2)]
                sst = ssps.tile([128, S], F32, tag="ss", name=f"{nm}p1")
                ps1 = [sst[:, 0:512], sst[:, 512:1024]]
                for d in range(nt_d):
                    for hh, pss in ((0, ps0), (1, ps1)):
                        for c in range(2):
                            nc.tensor.matmul(
                                pss[c],
                                proj_wsl[:, d, hh * 128:(hh + 1) * 128],
                                proj_x[:, d, c * 512:(c + 1) * 512],
                                start=(d == 0),
                                stop=(d == nt_d - 1),
                            )
                for hh, pss in ((0, ps0), (1, ps1)):
                    for c in range(2):
                        nc.scalar.activation(
                            out=dst[hh][:, c * 512:(c + 1) * 512], in_=pss[c],
                            func=ACT_IDENT, bias=bias_sb[:, hh:hh + 1],
                            scale=1.0)

            prologue_qk2(wk_sl[0], xk_sb, NT_DKV, bk_sb, kT, "kT")
            v4_alloc(0)
            # vproj g0: two rounds of 4 s-tiles, d-outer (2 pa + 1 ss tile)
            for rnd in range(2):
                pv_ps = [paps.tile([128, 512], F32, tag="pa",
                                   name=f"vp{rnd}_{i}") for i in range(2)]
                sst = ssps.tile([128, S], F32, tag="ss", name=f"vp{rnd}ss")
                pv_ps.append(sst[:, 0:512])
                pv_ps.append(sst[:, 512:1024])
                for d in range(NT_DKV):
                    for i in range(4):
                        st = rnd * 4 + i
                        nc.tensor.matmul(
                            pv_ps[i],
                            xv_sb[:, d, st * 128:(st + 1) * 128],
                            wv_sl[0][:, d, :],
                            start=(d == 0),
                            stop=(d == NT_DKV - 1),
                        )
                for i in range(4):
                    st = rnd * 4 + i
                    nc.vector.tensor_copy(
                        out=v4[0][:, st, :].rearrange(
                            "p (h c) -> p h c", c=VROW)[:, :, 0:HD],
                        in_=pv_ps[i].rearrange("p (h c) -> p h c", c=HD),
                    )
            prologue_qk2(wq_sl[0], xq_sb, NT_DQ, bq_sb, qT, "qT")

            # ---- main loop over heads; iter h projects heads h+2 ----
            for h in range(H):
                pt_t = [pts.tile([128, S], BF16, tag="pt", name=f"pt{h}_{sk}")
                        for sk in range(NT_S)]
                # weight slice j covers heads 2j/2j+1, first needed in iter
                # 2j-2; load at iter 2j-3 (slot j-2 frees at iter 2j-5).
                if h % 2 == 1 and (h + 3) // 2 < H // 2:
                    load_wqk2((h + 3) // 2)
                # wv group g first needed in iter 4g-3 (wv0/wv1 in prologue).
                if h == 2:
                    load_wv(2)
                elif h == 6:
                    load_wv(3)

                # vproj for group g spread over iters 4g-4..4g-1, two
                # s-tiles per iter (g0 was done in the prologue).
                vg, vst0, vnst = None, 0, 0
                if h <= 11:
                    vg, vst0, vnst = h // 4 + 1, (h % 4) * 2, 2
                    if vst0 == 0:
                        v4_alloc(vg)

                if h < H - 2:
                    scores_pair(h, pt_t, 0)
                    qproj(h + 2)
                    scores_pair(h, pt_t, 2)
                    kproj(h + 2)
                    scores_pair(h, pt_t, 4)
                    if vnst:
                        vproj_part(vg, vst0, vnst)
                    scores_pair(h, pt_t, 6)
                else:
                    # iters 14/15 have no projection work to hide exp()
                    # under, so pull in out-projection accumulation for
                    # chunks st=4,5 (e2c=0) using the idle pa psum tiles
                    # and separately-streamed Wo tiles (et 0..13).
                    if h == H - 2:
                        opull = [paps.tile([128, 512], F32, tag="pa",
                                           name=f"opull_{i}")
                                 for i in range(2)]
                        _CARRY["opull"] = opull
                    opull = _CARRY["opull"]

                    def opull_ets(e0, e1):
                        # uses the wo_pre tiles parked since the prologue
                        for et in range(e0, e1):
                            for i in range(2):
                                nc.tensor.matmul(
                                    opull[i],
                                    aoT_sb[:, et, (4 + i) * 128:(5 + i) * 128],
                                    wo_pre[et],
                                    start=(et == 0),
                                    stop=False,
                                )

                    base = 0 if h == H - 2 else 4
                    scores_pair(h, pt_t, 0)
                    opull_ets(base, base + 1)
                    scores_pair(h, pt_t, 2)
                    opull_ets(base + 1, base + 2)
                    scores_pair(h, pt_t, 4)
                    opull_ets(base + 2, base + 3)
                    scores_pair(h, pt_t, 6)
                    opull_ets(base + 3, base + 4)
                pv(h, pt_t)

            # ---- output projection: out[s, e2] = aoT.T @ WoT ----
            # 4 column passes of 8 chunks; Wo streamed per (e-tile, pass).
            for e2c in range(4):
                chunks = []
                for i in range(2):
                    t = ssps.tile([128, S], F32, tag="ss", name=f"oss_{e2c}_{i}")
                    chunks.append(t[:, 0:512])
                    chunks.append(t[:, 512:1024])
                for i in range(2):
                    if e2c == 0:
                        chunks.append(_CARRY["opull"][i])
                    else:
                        chunks.append(paps.tile([128, 512], F32, tag="pa",
                                                name=f"opa_{e2c}_{i}"))
                for i in range(2):
                    chunks.append(opps.tile([128, 512], F32, tag="op",
                                            name=f"oop_{e2c}_{i}"))
                def get_wo(et):
                    if e2c == 0 and et < len(wo_pre):
                        return wo_pre[et]
                    wo_t = wop.tile([128, 512], BF16, tag="wo",
                                    name=f"wo_{e2c}_{et}")
                    nc.gpsimd.dma_start(
                        out=wo_t,
                        in_=woT.ap()[et * 128:(et + 1) * 128,
                                     e2c * 512:(e2c + 1) * 512])
                    return wo_t

                # et-major streaming phase (et 0..7)
                for et in range(8):
                    wo_t = get_wo(et)
                    for st in range(NT_S):
                        if e2c == 0 and st in (4, 5):
                            continue  # accumulated during iters 14/15
                        nc.tensor.matmul(
                            chunks[st],
                            aoT_sb[:, et, st * 128:(st + 1) * 128],
                            wo_t,
                            start=(et == 0),
                            stop=False,
                        )
                # staggered tail: each chunk finishes its last 8 ets, then
                # copy+store immediately so completions pipeline out at a
                # spacing (~1.7us) above the copy+DMA drain rate
                wo_tail = {et: get_wo(et) for et in range(8, NT_E)}
                for st in range(NT_S):
                    for et in range(8, NT_E):
                        nc.tensor.matmul(
                            chunks[st],
                            aoT_sb[:, et, st * 128:(st + 1) * 128],
                            wo_tail[et],
                            start=False,
                            stop=(et == NT_E - 1),
                        )
                    ot = outsb.tile([128, 512], F32, tag="outt",
                                    name=f"ot_{e2c}_{st}")
                    if st % 2 == 0:
                        nc.vector.tensor_copy(out=ot, in_=chunks[st])
                        eng = nc.sync
                    else:
                        nc.scalar.copy(out=ot, in_=chunks[st])
                        eng = nc.scalar
                    eng.dma_start(
                        out=out.ap()[st * 128:(st + 1) * 128,
                                     e2c * 512:(e2c + 1) * 512],
                        in_=ot,
                    )

    nc.compile()
    return nc


def _get_nc():
    if "nc" not in _CACHED:
        _CACHED["nc"] = _build()
    return _CACHED["nc"]


def _numpy_reference(query, key, value, attention_mask,
                     Wq, bq, Wk, bk, Wv, bv, Wo, bo):
    # general fallback (only used when attention_mask isn't all ones)
    Bb, SQ, _ = query.shape
    SK = key.shape[1]
    q = query @ Wq.T + bq
    k = key @ Wk.T + bk
    v = value @ Wv.T + bv
    q = q.reshape(Bb, SQ, H, HD).transpose(0, 2, 1, 3)
    k = k.reshape(Bb, SK, H, HD).transpose(0, 2, 1, 3)
    v = v.reshape(Bb, SK, H, HD).transpose(0, 2, 1, 3)
    scores = np.einsum("bhqd,bhkd->bhqk", q, k) * (HD ** -0.5)
    scores = np.where(attention_mask[:, None, :, :] == 0,
                      np.float32(-1e10), scores)
    scores -= scores.max(-1, keepdims=True)
    p = np.exp(scores)
    p /= p.sum(-1, keepdims=True)
    o = np.einsum("bhqk,bhkd->bhqd", p, v)
    o = o.transpose(0, 2, 1, 3).reshape(Bb, SQ, E)
    return (o @ Wo.T + bo).astype(np.float32)


def _prepare_in_maps(inputs):
    query = np.asarray(inputs["query"], dtype=np.float32)
    key = np.asarray(inputs["key"], dtype=np.float32)
    value = np.asarray(inputs["value"], dtype=np.float32)
    Wq = np.asarray(inputs["Wq"], dtype=np.float32)
    bq = np.asarray(inputs["bq"], dtype=np.float32)
    Wk = np.asarray(inputs["Wk"], dtype=np.float32)
    bk = np.asarray(inputs["bk"], dtype=np.float32)
    Wv = np.asarray(inputs["Wv"], dtype=np.float32)
    Wo = np.asarray(inputs["Wo"], dtype=np.float32)

    scale = np.float32(HD ** -0.5)
    bf = ml_dtypes.bfloat16
    wqT = np.ascontiguousarray((Wq.T * scale).astype(bf))
    wkT = np.ascontiguousarray(Wk.T.astype(bf))
    wvT = np.ascontiguousarray(Wv.T.astype(bf))
    woT = np.ascontiguousarray(Wo.T.astype(bf))
    bq_s = (bq * scale).astype(np.float32)

    in_maps = []
    for b in range(B):
        in_maps.append({
            "xqT": np.ascontiguousarray(query[b].T.astype(bf)),
            "xkT": np.ascontiguousarray(key[b].T.astype(bf)),
            "xvT": np.ascontiguousarray(value[b].T.astype(bf)),
            "wqT": wqT, "wkT": wkT, "wvT": wvT, "woT": woT,
            "bq": bq_s, "bk": bk.astype(np.float32),
        })
    return in_maps


def run_on_device(inputs, **spmd_kwargs):
    """Run the bass kernel; returns (out [B,S,E] f32, BassKernelResults)."""
    in_maps = _prepare_in_maps(inputs)
    Wo = np.asarray(inputs["Wo"], dtype=np.float64)
    bv = np.asarray(inputs["bv"], dtype=np.float64)
    bo = np.asarray(inputs["bo"], dtype=np.float64)
    bo_eff = (Wo @ bv + bo).astype(np.float32)
    res = run_bass_kernel_spmd(_get_nc(), in_maps,
                               core_ids=list(range(B)), **spmd_kwargs)
    out = np.stack([res.results[b]["out"] for b in range(B)], axis=0)
    return (out + bo_eff).astype(np.float32), res


def kernel(**inputs):
    mask = np.asarray(inputs["attention_mask"])
    if not mask.all():
        return _numpy_reference(
            np.asarray(inputs["query"], dtype=np.float32),
            np.asarray(inputs["key"], dtype=np.float32),
            np.asarray(inputs["value"], dtype=np.float32), mask,
            np.asarray(inputs["Wq"], dtype=np.float32),
            np.asarray(inputs["bq"], dtype=np.float32),
            np.asarray(inputs["Wk"], dtype=np.float32),
            np.asarray(inputs["bk"], dtype=np.float32),
            np.asarray(inputs["Wv"], dtype=np.float32),
            np.asarray(inputs["bv"], dtype=np.float32),
            np.asarray(inputs["Wo"], dtype=np.float32),
            np.asarray(inputs["bo"], dtype=np.float32))
    out, _ = run_on_device(inputs)
    return out


# revision 48
# speedup vs baseline: 1.4028x; 1.0010x over previous
"""CrossAttention kernel for 8 Trainium2 NeuronCores.

Problem (hardcoded): B=8, SQ=SK=1024, Q_DIM=2048, KV_DIM=1024, E_DIM=2048,
H=16 heads, HD=128.  out = softmax((X_q Wq^T + bq)(X_k Wk^T + bk)^T / sqrt(HD))
                            @ (X_v Wv^T + bv) @ Wo^T + bo

Sharding: data-parallel over batch — each of the 8 cores computes one batch
element end-to-end; no collectives.

Per-core dataflow (all matmuls bf16, f32 PSUM accumulation), software-
pipelined per head so the scalar engine's exp() hides under the next head's
projections:

  iter h: [scores(h) tiles interleaved with qproj(h+1)/kproj(h+1)/vproj part]
          then PV(h) -> ao_h -> DMA-transpose into aoT[:, h, :].
  - qT/kT produced in [e, s] layout (weight stationary).
  - v produced directly in [s, e] layout (xvT stationary, wv moving) with a
    ones column per head block => softmax denominators ride along as PV
    output column 128.  bv is folded into bo on the host (softmax rows sum
    to 1, so + bv passes through attention exactly).
  - out = aoT.T @ WoT accumulated over e-tiles with Wo streamed from DRAM;
    chunks copied+stored as they complete.
"""

import sys

sys.path.insert(0, "/opt/trn_rl_repo")

import numpy as np
import ml_dtypes

import concourse.tile as tile
from concourse import bacc
import concourse.mybir as mybir
from concourse.bass_utils import run_bass_kernel_spmd

F32 = mybir.dt.float32
BF16 = mybir.dt.bfloat16
ACT_IDENT = mybir.ActivationFunctionType.Identity
ACT_COPY = mybir.ActivationFunctionType.Copy
ACT_EXP = mybir.ActivationFunctionType.Exp

B = 8
S = 1024          # SQ == SK
DQ = 2048         # query input dim
DKV = 1024        # key/value input dim
E = 2048          # embed dim
H = 16            # heads
HD = 128          # head dim
NT_S = S // 128   # 8 seq tiles
NT_E = E // 128   # 16 e tiles (== heads)
NT_DQ = DQ // 128
NT_DKV = DKV // 128
VROW = HD + 1     # head block in v group incl. ones column

_CACHED = {}


def _build():
    nc = bacc.Bacc("TRN2", target_bir_lowering=False, debug=False)

    xqT = nc.dram_tensor("xqT", [DQ, S], BF16, kind="ExternalInput")
    xkT = nc.dram_tensor("xkT", [DKV, S], BF16, kind="ExternalInput")
    xvT = nc.dram_tensor("xvT", [DKV, S], BF16, kind="ExternalInput")
    wqT = nc.dram_tensor("wqT", [DQ, E], BF16, kind="ExternalInput")
    wkT = nc.dram_tensor("wkT", [DKV, E], BF16, kind="ExternalInput")
    wvT = nc.dram_tensor("wvT", [DKV, E], BF16, kind="ExternalInput")
    woT = nc.dram_tensor("woT", [E, E], BF16, kind="ExternalInput")
    bq = nc.dram_tensor("bq", [E], F32, kind="ExternalInput")
    bk = nc.dram_tensor("bk", [E], F32, kind="ExternalInput")
    out = nc.dram_tensor("out", [S, E], F32, kind="ExternalOutput")

    xqT_r = xqT.rearrange("(t p) s -> p t s", p=128)
    xkT_r = xkT.rearrange("(t p) s -> p t s", p=128)
    xvT_r = xvT.rearrange("(t p) s -> p t s", p=128)
    wqT_r = wqT.rearrange("(t p) e -> p t e", p=128)
    wkT_r = wkT.rearrange("(t p) e -> p t e", p=128)
    wvT_r = wvT.rearrange("(t p) e -> p t e", p=128)

    with tile.TileContext(nc) as tc:
        with (
            tc.tile_pool(name="persist", bufs=1) as persist,
            tc.tile_pool(name="qk", bufs=3) as qkp,
            tc.tile_pool(name="v4p", bufs=2) as v4p,
            tc.tile_pool(name="pts", bufs=10) as pts,
            tc.tile_pool(name="aohp", bufs=2) as aohp,
            tc.tile_pool(name="wqp", bufs=2) as wqp,
            tc.tile_pool(name="wkp", bufs=2) as wkp,
            tc.tile_pool(name="wvp", bufs=2) as wvp,
            tc.tile_pool(name="wop", bufs=10) as wop,
            tc.tile_pool(name="outsb", bufs=3) as outsb,
            tc.tile_pool(name="small", bufs=4) as small,
            tc.tile_pool(name="ssps", bufs=2, space="PSUM") as ssps,
            tc.tile_pool(name="paps", bufs=2, space="PSUM") as paps,
            tc.tile_pool(name="opps", bufs=2, space="PSUM") as opps,
        ):
            # ---- resident inputs / constants ----
            bq_sb = persist.tile([128, NT_E], F32, tag="bq")
            bk_sb = persist.tile([128, NT_E], F32, tag="bk")
            nc.gpsimd.dma_start(out=bq_sb, in_=bq.rearrange("(t p) -> p t", p=128))
            nc.gpsimd.dma_start(out=bk_sb, in_=bk.rearrange("(t p) -> p t", p=128))

            xq_sb = persist.tile([128, NT_DQ, S], BF16, tag="xq")
            xk_sb = persist.tile([128, NT_DKV, S], BF16, tag="xk")
            xv_sb = persist.tile([128, NT_DKV, S], BF16, tag="xv")
            aoT_sb = persist.tile([128, NT_E, S], BF16, tag="aoT")

            # weight slices (wq/wk cover 2 heads per slice)
            wq_sl = {}
            wk_sl = {}
            wv_sl = {}

            def load_wqk2(j, eng=None):  # heads 2j, 2j+1
                eng = eng or nc.scalar
                wq_sl[j] = wqp.tile([128, NT_DQ, 256], BF16, tag="wq",
                                    name=f"wq2_{j}")
                eng.dma_start(
                    out=wq_sl[j], in_=wqT_r[:, :, 2 * j * 128:(2 * j + 2) * 128])
                wk_sl[j] = wkp.tile([128, NT_DKV, 256], BF16, tag="wk",
                                    name=f"wk2_{j}")
                eng.dma_start(
                    out=wk_sl[j], in_=wkT_r[:, :, 2 * j * 128:(2 * j + 2) * 128])

            def load_wv(g, eng=None):  # heads 4g..4g+3
                eng = eng or nc.scalar
                wv_sl[g] = wvp.tile([128, NT_DKV, 512], BF16, tag="wv",
                                    name=f"wv_{g}")
                eng.dma_start(
                    out=wv_sl[g], in_=wvT_r[:, :, 4 * g * 128:(4 * g + 4) * 128])

            # Prologue DMAs: ALL on the sync queue, in exact consumption
            # order (the DMA device serves transfers in arrival order, and
            # cross-queue ordering is uncontrolled): kproj needs wk+xk,
            # then vproj g0 needs wv+xv, then qproj needs wq+xq.
            # first weight halves go on the scalar queue so their DGE
            # latency overlaps the sync-queue xk stream
            wk_sl[0] = wkp.tile([128, NT_DKV, 256], BF16, tag="wk", name="wk2_0")
            nc.scalar.dma_start(out=wk_sl[0][:, 0:4, :], in_=wkT_r[:, 0:4, 0:256])
            nc.scalar.dma_start(out=wk_sl[0][:, 4:8, :], in_=wkT_r[:, 4:8, 0:256])
            for d in range(NT_DKV):
                nc.sync.dma_start(out=xk_sb[:, d, :], in_=xkT_r[:, d, :])
            wv_sl[0] = wvp.tile([128, NT_DKV, 512], BF16, tag="wv", name="wv_0")
            nc.sync.dma_start(out=wv_sl[0][:, 0:4, :], in_=wvT_r[:, 0:4, 0:512])
            for d in range(4):
                nc.sync.dma_start(out=xv_sb[:, d, :], in_=xvT_r[:, d, :])
            nc.sync.dma_start(out=wv_sl[0][:, 4:8, :], in_=wvT_r[:, 4:8, 0:512])
            for d in range(4, NT_DKV):
                nc.sync.dma_start(out=xv_sb[:, d, :], in_=xvT_r[:, d, :])
            wq_sl[0] = wqp.tile([128, NT_DQ, 256], BF16, tag="wq", name="wq2_0")
            nc.sync.dma_start(out=wq_sl[0], in_=wqT_r[:, :, 0:256])
            for d in range(NT_DQ):
                nc.sync.dma_start(out=xq_sb[:, d, :], in_=xqT_r[:, d, :])
            load_wqk2(1, eng=nc.sync)
            load_wv(1, eng=nc.sync)
            # prefetch first out-projection weight tiles (parked until the
            # epilogue; also throttles the gpsimd wo stream via pool slots)
            wo_pre = []
            for et in range(8):
                wo_t = wop.tile([128, 512], BF16, tag="wo", name=f"wo_0_{et}")
                nc.sync.dma_start(
                    out=wo_t, in_=woT.ap()[et * 128:(et + 1) * 128, 0:512])
                wo_pre.append(wo_t)

            qT = {}
            kT = {}
            v4 = {}
            _CARRY = {}

            def qproj(h):
                """qT[h] [e128, s] <- sum_d wq-block.T @ xq (2 psum chunks)."""
                sl = wq_sl[h // 2]
                hoff = (h % 2) * 128
                qT[h] = qkp.tile([128, S], BF16, tag="qT", name=f"qT_{h}")
                for c in range(2):
                    ps = paps.tile([128, 512], F32, tag="pa", name=f"qps_{h}_{c}")
                    for d in range(NT_DQ):
                        nc.tensor.matmul(
                            ps,
                            sl[:, d, hoff:hoff + 128],
                            xq_sb[:, d, c * 512:(c + 1) * 512],
                            start=(d == 0),
                            stop=(d == NT_DQ - 1),
                        )
                    nc.scalar.activation(
                        out=qT[h][:, c * 512:(c + 1) * 512], in_=ps,
                        func=ACT_IDENT, bias=bq_sb[:, h:h + 1], scale=1.0)

            def kproj(h):
                sl = wk_sl[h // 2]
                hoff = (h % 2) * 128
                kT[h] = qkp.tile([128, S], BF16, tag="kT", name=f"kT_{h}")
                for c in range(2):
                    ps = paps.tile([128, 512], F32, tag="pa", name=f"kps_{h}_{c}")
                    for d in range(NT_DKV):
                        nc.tensor.matmul(
                            ps,
                            sl[:, d, hoff:hoff + 128],
                            xk_sb[:, d, c * 512:(c + 1) * 512],
                            start=(d == 0),
                            stop=(d == NT_DKV - 1),
                        )
                    nc.scalar.activation(
                        out=kT[h][:, c * 512:(c + 1) * 512], in_=ps,
                        func=ACT_IDENT, bias=bk_sb[:, h:h + 1], scale=1.0)

            def v4_alloc(g):
                v4[g] = v4p.tile([128, NT_S, 4 * VROW], BF16, tag="v4",
                                 name=f"v4_{g}")
                nc.vector.memset(
                    v4[g].rearrange("p t (h c) -> p t h c", c=VROW)
                    [:, :, :, HD:], 1.0)

            def vproj_part(g, st0, nst):
                """v4[g][:, st, :] <- (xv st-block).T @ wv-group, s-tiles
                st0..st0+nst-1; heads 4g..4g+3 with ones columns."""
                sl = wv_sl[g]
                for st in range(st0, st0 + nst):
                    ps = paps.tile([128, 512], F32, tag="pa",
                                   name=f"vps_{g}_{st}")
                    for d in range(NT_DKV):
                        nc.tensor.matmul(
                            ps,
                            xv_sb[:, d, st * 128:(st + 1) * 128],
                            sl[:, d, :],
                            start=(d == 0),
                            stop=(d == NT_DKV - 1),
                        )
                    nc.vector.tensor_copy(
                        out=v4[g][:, st, :].rearrange(
                            "p (h c) -> p h c", c=VROW)[:, :, 0:HD],
                        in_=ps.rearrange("p (h c) -> p h c", c=HD),
                    )

            PT = {}

            def scores_pair(h, s0):
                """two score tiles s0, s0+1: scoresT[sk,sq] -> exp -> pt."""
                for sk in (s0, s0 + 1):
                    pt_sk = pts.tile([128, S], BF16, tag="pt",
                                     name=f"pt{h}_{sk}")
                    PT.setdefault(h, {})[sk] = pt_sk
                    ss = ssps.tile([128, S], F32, tag="ss",
                                   name=f"ss_{h}_{sk}")
                    for c in range(2):
                        nc.tensor.matmul(
                            ss[:, c * 512:(c + 1) * 512],
                            kT[h][:, sk * 128:(sk + 1) * 128],
                            qT[h][:, c * 512:(c + 1) * 512],
                            start=True,
                            stop=True,
                        )
                    nc.scalar.activation(
                        out=pt_sk, in_=ss, func=ACT_EXP, bias=0.0, scale=1.0)

            def pv(h):
                pt_t = PT[h]
                g, hig = h // 4, h % 4
                ao_h = aohp.tile([128, NT_S, HD], BF16, tag="aoh",
                                 name=f"aoh_{h}")
                # 4 rotating accumulator slots: 2 op-pool tiles + both banks
                # of one ss tile (scores(h) has fully drained through exp by
                # now) — wide enough that the recip+scale drain never stalls
                # the PV matmul stream.
                ssa = ssps.tile([128, S], F32, tag="ss", name=f"pvss_{h}")
                opa = opps.tile([128, 512], F32, tag="op", name=f"opa_{h}")
                opb = opps.tile([128, 512], F32, tag="op", name=f"opb_{h}")
                chunk4 = [opa, opb, ssa[:, 0:512], ssa[:, 512:1024]]
                for st in range(NT_S):
                    op = chunk4[st % 4]
                    for sk in range(NT_S):
                        nc.tensor.matmul(
                            op[:, 0:VROW],
                            pt_t[sk][:, st * 128:(st + 1) * 128],
                            v4[g][:, sk, hig * VROW:(hig + 1) * VROW],
                            start=(sk == 0),
                            stop=(sk == NT_S - 1),
                        )
                    rec = small.tile([128, 1], F32, tag="rec",
                                     name=f"rec_{h}_{st}")
                    nc.vector.reciprocal(out=rec, in_=op[:, HD:VROW])
                    nc.vector.tensor_scalar_mul(
                        ao_h[:, st, :], op[:, 0:HD], rec)
                nc.sync.dma_start_transpose(
                    out=aoT_sb[:, h, :].rearrange("p (t c) -> p t c", c=128),
                    in_=ao_h)

            # ---- prologue compute, d-outer interleaved so PE consumption
            # rate (4 mms per d-tile) stays behind the DMA arrival rate ----
            def prologue_qk2(proj_wsl, proj_x, nt_d, bias_sb, dst, nm):
                dst[0] = qkp.tile([128, S], BF16, tag=nm, name=f"{nm}_0")
                dst[1] = qkp.tile([128, S], BF16, tag=nm, name=f"{nm}_1")
                ps0 = [paps.tile([128, 512], F32, tag="pa",
                                 name=f"{nm}p0_{c}") for c in range(2)]
                sst = ssps.tile([128, S], F32, tag="ss", name=f"{nm}p1")
                ps1 = [sst[:, 0:512], sst[:, 512:1024]]
                for d in range(nt_d):
                    for hh, pss in ((0, ps0), (1, ps1)):
                        for c in range(2):
                            nc.tensor.matmul(
                                pss[c],
                                proj_wsl[:, d, hh * 128:(hh + 1) * 128],
                                proj_x[:, d, c * 512:(c + 1) * 512],
                                start=(d == 0),
                                stop=(d == nt_d - 1),
                            )
                for hh, pss in ((0, ps0), (1, ps1)):
                    for c in range(2):
                        nc.scalar.activation(
                            out=dst[hh][:, c * 512:(c + 1) * 512], in_=pss[c],
                            func=ACT_IDENT, bias=bias_sb[:, hh:hh + 1],
                            scale=1.0)

            prologue_qk2(wk_sl[0], xk_sb, NT_DKV, bk_sb, kT, "kT")
            v4_alloc(0)
            # vproj g0: two rounds of 4 s-tiles, d-outer (2 pa + 1 ss tile)
            for rnd in range(2):
                pv_ps = [paps.tile([128, 512], F32, tag="pa",
                                   name=f"vp{rnd}_{i}") for i in range(2)]
                sst = ssps.tile([128, S], F32, tag="ss", name=f"vp{rnd}ss")
                pv_ps.append(sst[:, 0:512])
                pv_ps.append(sst[:, 512:1024])
                for d in range(NT_DKV):
                    for i in range(4):
                        st = rnd * 4 + i
                        nc.tensor.matmul(
                            pv_ps[i],
                            xv_sb[:, d, st * 128:(st + 1) * 128],
                            wv_sl[0][:, d, :],
                            start=(d == 0),
                            stop=(d == NT_DKV - 1),
                        )
                for i in range(4):
                    st = rnd * 4 + i
                    nc.vector.tensor_copy(
                        out=v4[0][:, st, :].rearrange(
                            "p (h c) -> p h c", c=VROW)[:, :, 0:HD],
                        in_=pv_ps[i].rearrange("p (h c) -> p h c", c=HD),
                    )
            prologue_qk2(wq_sl[0], xq_sb, NT_DQ, bq_sb, qT, "qT")

            # ---- main loop over heads; iter h projects heads h+2 ----
            for h in range(H):
                # weight slice j covers heads 2j/2j+1, first needed in iter
                # 2j-2; load at iter 2j-3 (slot j-2 frees at iter 2j-5).
                if h % 2 == 1 and (h + 3) // 2 < H // 2:
                    load_wqk2((h + 3) // 2)
                # wv group g first needed in iter 4g-3 (wv0/wv1 in prologue).
                if h == 2:
                    load_wv(2)
                elif h == 6:
                    load_wv(3)

                # vproj for group g spread over iters 4g-4..4g-1, two
                # s-tiles per iter (g0 was done in the prologue).
                vg, vst0, vnst = None, 0, 0
                if h <= 11:
                    vg, vst0, vnst = h // 4 + 1, (h % 4) * 2, 2
                    if vst0 == 0:
                        v4_alloc(vg)

                if h < H - 2:
                    scores_pair(h, 0)
                    qproj(h + 2)
                    scores_pair(h, 2)
                    kproj(h + 2)
                    scores_pair(h, 4)
                    if vnst:
                        vproj_part(vg, vst0, vnst)
                    scores_pair(h, 6)
                    if h == 13:
                        # head 14's first score pair computed early so its
                        # exps drain on ACT during this iteration's slack
                        scores_pair(14, 0)
                else:
                    # iters 14/15 have no projection work to hide exp()
                    # under, so pull in out-projection accumulation for
                    # chunks st=4,5 (e2c=0) using the idle pa psum tiles
                    # and separately-streamed Wo tiles (et 0..13).
                    if h == H - 2:
                        opull = [paps.tile([128, 512], F32, tag="pa",
                                           name=f"opull_{i}")
                                 for i in range(2)]
                        _CARRY["opull"] = opull
                    opull = _CARRY["opull"]

                    def opull_ets(e0, e1):
                        # uses the wo_pre tiles parked since the prologue
                        for et in range(e0, e1):
                            for i in range(2):
                                nc.tensor.matmul(
                                    opull[i],
                                    aoT_sb[:, et, (4 + i) * 128:(5 + i) * 128],
                                    wo_pre[et],
                                    start=(et == 0),
                                    stop=False,
                                )

                    # score pair 0 was computed in the previous iteration;
                    # head 15's pair 0 is pulled into iter 14 here.
                    base = 0 if h == H - 2 else 4
                    scores_pair(h, 2)
                    opull_ets(base, base + 1)
                    scores_pair(h, 4)
                    opull_ets(base + 1, base + 2)
                    scores_pair(h, 6)
                    opull_ets(base + 2, base + 3)
                    if h == H - 2:
                        scores_pair(15, 0)
                    opull_ets(base + 3, base + 4)
                pv(h)

            # ---- output projection: out[s, e2] = aoT.T @ WoT ----
            # 4 column passes of 8 chunks; Wo streamed per (e-tile, pass).
            for e2c in range(4):
                chunks = []
                for i in range(2):
                    t = ssps.tile([128, S], F32, tag="ss", name=f"oss_{e2c}_{i}")
                    chunks.append(t[:, 0:512])
                    chunks.append(t[:, 512:1024])
                for i in range(2):
                    if e2c == 0:
                        chunks.append(_CARRY["opull"][i])
                    else:
                        chunks.append(paps.tile([128, 512], F32, tag="pa",
                                                name=f"opa_{e2c}_{i}"))
                for i in range(2):
                    chunks.append(opps.tile([128, 512], F32, tag="op",
                                            name=f"oop_{e2c}_{i}"))
                def get_wo(et):
                    if e2c == 0 and et < len(wo_pre):
                        return wo_pre[et]
                    wo_t = wop.tile([128, 512], BF16, tag="wo",
                                    name=f"wo_{e2c}_{et}")
                    nc.gpsimd.dma_start(
                        out=wo_t,
                        in_=woT.ap()[et * 128:(et + 1) * 128,
                                     e2c * 512:(e2c + 1) * 512])
                    return wo_t

                # et-major streaming phase (et 0..7)
                for et in range(8):
                    wo_t = get_wo(et)
                    for st in range(NT_S):
                        if e2c == 0 and st in (4, 5):
                            continue  # accumulated during iters 14/15
                        nc.tensor.matmul(
                            chunks[st],
                            aoT_sb[:, et, st * 128:(st + 1) * 128],
                            wo_t,
                            start=(et == 0),
                            stop=False,
                        )
                # staggered tail: each chunk finishes its last 8 ets, then
                # copy+store immediately so completions pipeline out at a
                # spacing (~1.7us) above the copy+DMA drain rate
                wo_tail = {et: get_wo(et) for et in range(8, NT_E)}
                for st in range(NT_S):
                    for et in range(8, NT_E):
                        nc.tensor.matmul(
                            chunks[st],
                            aoT_sb[:, et, st * 128:(st + 1) * 128],
                            wo_tail[et],
                            start=False,
                            stop=(et == NT_E - 1),
                        )
                    ot = outsb.tile([128, 512], F32, tag="outt",
                                    name=f"ot_{e2c}_{st}")
                    if st % 2 == 0:
                        nc.vector.tensor_copy(out=ot, in_=chunks[st])
                        eng = nc.sync
                    else:
                        nc.scalar.copy(out=ot, in_=chunks[st])
                        eng = nc.scalar
                    eng.dma_start(
                        out=out.ap()[st * 128:(st + 1) * 128,
                                     e2c * 512:(e2c + 1) * 512],
                        in_=ot,
                    )

    nc.compile()
    return nc


def _get_nc():
    if "nc" not in _CACHED:
        _CACHED["nc"] = _build()
    return _CACHED["nc"]


def _numpy_reference(query, key, value, attention_mask,
                     Wq, bq, Wk, bk, Wv, bv, Wo, bo):
    # general fallback (only used when attention_mask isn't all ones)
    Bb, SQ, _ = query.shape
    SK = key.shape[1]
    q = query @ Wq.T + bq
    k = key @ Wk.T + bk
    v = value @ Wv.T + bv
    q = q.reshape(Bb, SQ, H, HD).transpose(0, 2, 1, 3)
    k = k.reshape(Bb, SK, H, HD).transpose(0, 2, 1, 3)
    v = v.reshape(Bb, SK, H, HD).transpose(0, 2, 1, 3)
    scores = np.einsum("bhqd,bhkd->bhqk", q, k) * (HD ** -0.5)
    scores = np.where(attention_mask[:, None, :, :] == 0,
                      np.float32(-1e10), scores)
    scores -= scores.max(-1, keepdims=True)
    p = np.exp(scores)
    p /= p.sum(-1, keepdims=True)
    o = np.einsum("bhqk,bhkd->bhqd", p, v)
    o = o.transpose(0, 2, 1, 3).reshape(Bb, SQ, E)
    return (o @ Wo.T + bo).astype(np.float32)


def _prepare_in_maps(inputs):
    query = np.asarray(inputs["query"], dtype=np.float32)
    key = np.asarray(inputs["key"], dtype=np.float32)
    value = np.asarray(inputs["value"], dtype=np.float32)
    Wq = np.asarray(inputs["Wq"], dtype=np.float32)
    bq = np.asarray(inputs["bq"], dtype=np.float32)
    Wk = np.asarray(inputs["Wk"], dtype=np.float32)
    bk = np.asarray(inputs["bk"], dtype=np.float32)
    Wv = np.asarray(inputs["Wv"], dtype=np.float32)
    Wo = np.asarray(inputs["Wo"], dtype=np.float32)

    scale = np.float32(HD ** -0.5)
    bf = ml_dtypes.bfloat16
    wqT = np.ascontiguousarray((Wq.T * scale).astype(bf))
    wkT = np.ascontiguousarray(Wk.T.astype(bf))
    wvT = np.ascontiguousarray(Wv.T.astype(bf))
    woT = np.ascontiguousarray(Wo.T.astype(bf))
    bq_s = (bq * scale).astype(np.float32)

    in_maps = []
    for b in range(B):
        in_maps.append({
            "xqT": np.ascontiguousarray(query[b].T.astype(bf)),
            "xkT": np.ascontiguousarray(key[b].T.astype(bf)),
            "xvT": np.ascontiguousarray(value[b].T.astype(bf)),
            "wqT": wqT, "wkT": wkT, "wvT": wvT, "woT": woT,
            "bq": bq_s, "bk": bk.astype(np.float32),
        })
    return in_maps


def run_on_device(inputs, **spmd_kwargs):
    """Run the bass kernel; returns (out [B,S,E] f32, BassKernelResults)."""
    in_maps = _prepare_in_maps(inputs)
    Wo = np.asarray(inputs["Wo"], dtype=np.float64)
    bv = np.asarray(inputs["bv"], dtype=np.float64)
    bo = np.asarray(inputs["bo"], dtype=np.float64)
    bo_eff = (Wo @ bv + bo).astype(np.float32)
    res = run_bass_kernel_spmd(_get_nc(), in_maps,
                               core_ids=list(range(B)), **spmd_kwargs)
    out = np.stack([res.results[b]["out"] for b in range(B)], axis=0)
    return (out + bo_eff).astype(np.float32), res


def kernel(**inputs):
    mask = np.asarray(inputs["attention_mask"])
    if not mask.all():
        return _numpy_reference(
            np.asarray(inputs["query"], dtype=np.float32),
            np.asarray(inputs["key"], dtype=np.float32),
            np.asarray(inputs["value"], dtype=np.float32), mask,
            np.asarray(inputs["Wq"], dtype=np.float32),
            np.asarray(inputs["bq"], dtype=np.float32),
            np.asarray(inputs["Wk"], dtype=np.float32),
            np.asarray(inputs["bk"], dtype=np.float32),
            np.asarray(inputs["Wv"], dtype=np.float32),
            np.asarray(inputs["bv"], dtype=np.float32),
            np.asarray(inputs["Wo"], dtype=np.float32),
            np.asarray(inputs["bo"], dtype=np.float32))
    out, _ = run_on_device(inputs)
    return out


# revision 49
# speedup vs baseline: 1.4076x; 1.0034x over previous
"""CrossAttention kernel for 8 Trainium2 NeuronCores.

Problem (hardcoded): B=8, SQ=SK=1024, Q_DIM=2048, KV_DIM=1024, E_DIM=2048,
H=16 heads, HD=128.  out = softmax((X_q Wq^T + bq)(X_k Wk^T + bk)^T / sqrt(HD))
                            @ (X_v Wv^T + bv) @ Wo^T + bo

Sharding: data-parallel over batch — each of the 8 cores computes one batch
element end-to-end; no collectives.

Per-core dataflow (all matmuls bf16, f32 PSUM accumulation), software-
pipelined per head so the scalar engine's exp() hides under the next head's
projections:

  iter h: [scores(h) tiles interleaved with qproj(h+1)/kproj(h+1)/vproj part]
          then PV(h) -> ao_h -> DMA-transpose into aoT[:, h, :].
  - qT/kT produced in [e, s] layout (weight stationary).
  - v produced directly in [s, e] layout (xvT stationary, wv moving) with a
    ones column per head block => softmax denominators ride along as PV
    output column 128.  bv is folded into bo on the host (softmax rows sum
    to 1, so + bv passes through attention exactly).
  - out = aoT.T @ WoT accumulated over e-tiles with Wo streamed from DRAM;
    chunks copied+stored as they complete.
"""

import sys

sys.path.insert(0, "/opt/trn_rl_repo")

import numpy as np
import ml_dtypes

import concourse.tile as tile
from concourse import bacc
import concourse.mybir as mybir
from concourse.bass_utils import run_bass_kernel_spmd

F32 = mybir.dt.float32
BF16 = mybir.dt.bfloat16
ACT_IDENT = mybir.ActivationFunctionType.Identity
ACT_COPY = mybir.ActivationFunctionType.Copy
ACT_EXP = mybir.ActivationFunctionType.Exp

B = 8
S = 1024          # SQ == SK
DQ = 2048         # query input dim
DKV = 1024        # key/value input dim
E = 2048          # embed dim
H = 16            # heads
HD = 128          # head dim
NT_S = S // 128   # 8 seq tiles
NT_E = E // 128   # 16 e tiles (== heads)
NT_DQ = DQ // 128
NT_DKV = DKV // 128
VROW = HD + 1     # head block in v group incl. ones column

_CACHED = {}


def _build():
    nc = bacc.Bacc("TRN2", target_bir_lowering=False, debug=False)

    xqT = nc.dram_tensor("xqT", [DQ, S], BF16, kind="ExternalInput")
    xkT = nc.dram_tensor("xkT", [DKV, S], BF16, kind="ExternalInput")
    xvT = nc.dram_tensor("xvT", [DKV, S], BF16, kind="ExternalInput")
    wqT = nc.dram_tensor("wqT", [DQ, E], BF16, kind="ExternalInput")
    wkT = nc.dram_tensor("wkT", [DKV, E], BF16, kind="ExternalInput")
    wvT = nc.dram_tensor("wvT", [DKV, E], BF16, kind="ExternalInput")
    woT = nc.dram_tensor("woT", [E, E], BF16, kind="ExternalInput")
    bq = nc.dram_tensor("bq", [E], F32, kind="ExternalInput")
    bk = nc.dram_tensor("bk", [E], F32, kind="ExternalInput")
    out = nc.dram_tensor("out", [S, E], F32, kind="ExternalOutput")

    xqT_r = xqT.rearrange("(t p) s -> p t s", p=128)
    xkT_r = xkT.rearrange("(t p) s -> p t s", p=128)
    xvT_r = xvT.rearrange("(t p) s -> p t s", p=128)
    wqT_r = wqT.rearrange("(t p) e -> p t e", p=128)
    wkT_r = wkT.rearrange("(t p) e -> p t e", p=128)
    wvT_r = wvT.rearrange("(t p) e -> p t e", p=128)

    with tile.TileContext(nc) as tc:
        with (
            tc.tile_pool(name="persist", bufs=1) as persist,
            tc.tile_pool(name="qk", bufs=3) as qkp,
            tc.tile_pool(name="v4p", bufs=2) as v4p,
            tc.tile_pool(name="pts", bufs=10) as pts,
            tc.tile_pool(name="aohp", bufs=2) as aohp,
            tc.tile_pool(name="wqp", bufs=2) as wqp,
            tc.tile_pool(name="wkp", bufs=2) as wkp,
            tc.tile_pool(name="wvp", bufs=2) as wvp,
            tc.tile_pool(name="wop", bufs=10) as wop,
            tc.tile_pool(name="outsb", bufs=3) as outsb,
            tc.tile_pool(name="small", bufs=4) as small,
            tc.tile_pool(name="ssps", bufs=2, space="PSUM") as ssps,
            tc.tile_pool(name="paps", bufs=2, space="PSUM") as paps,
            tc.tile_pool(name="opps", bufs=2, space="PSUM") as opps,
        ):
            # ---- resident inputs / constants ----
            bq_sb = persist.tile([128, NT_E], F32, tag="bq")
            bk_sb = persist.tile([128, NT_E], F32, tag="bk")
            nc.gpsimd.dma_start(out=bq_sb, in_=bq.rearrange("(t p) -> p t", p=128))
            nc.gpsimd.dma_start(out=bk_sb, in_=bk.rearrange("(t p) -> p t", p=128))

            xq_sb = persist.tile([128, NT_DQ, S], BF16, tag="xq")
            xk_sb = persist.tile([128, NT_DKV, S], BF16, tag="xk")
            xv_sb = persist.tile([128, NT_DKV, S], BF16, tag="xv")
            aoT_sb = persist.tile([128, NT_E, S], BF16, tag="aoT")

            # weight slices (wq/wk cover 2 heads per slice)
            wq_sl = {}
            wk_sl = {}
            wv_sl = {}

            def load_wqk2(j, eng=None):  # heads 2j, 2j+1
                eng = eng or nc.scalar
                wq_sl[j] = wqp.tile([128, NT_DQ, 256], BF16, tag="wq",
                                    name=f"wq2_{j}")
                eng.dma_start(
                    out=wq_sl[j], in_=wqT_r[:, :, 2 * j * 128:(2 * j + 2) * 128])
                wk_sl[j] = wkp.tile([128, NT_DKV, 256], BF16, tag="wk",
                                    name=f"wk2_{j}")
                eng.dma_start(
                    out=wk_sl[j], in_=wkT_r[:, :, 2 * j * 128:(2 * j + 2) * 128])

            def load_wv(g, eng=None):  # heads 4g..4g+3
                eng = eng or nc.scalar
                wv_sl[g] = wvp.tile([128, NT_DKV, 512], BF16, tag="wv",
                                    name=f"wv_{g}")
                eng.dma_start(
                    out=wv_sl[g], in_=wvT_r[:, :, 4 * g * 128:(4 * g + 4) * 128])

            # Prologue DMAs: ALL on the sync queue, in exact consumption
            # order (the DMA device serves transfers in arrival order, and
            # cross-queue ordering is uncontrolled): kproj needs wk+xk,
            # then vproj g0 needs wv+xv, then qproj needs wq+xq.
            # first weight halves go on the scalar queue so their DGE
            # latency overlaps the sync-queue xk stream
            wk_sl[0] = wkp.tile([128, NT_DKV, 256], BF16, tag="wk", name="wk2_0")
            nc.scalar.dma_start(out=wk_sl[0][:, 0:4, :], in_=wkT_r[:, 0:4, 0:256])
            nc.scalar.dma_start(out=wk_sl[0][:, 4:8, :], in_=wkT_r[:, 4:8, 0:256])
            for d in range(NT_DKV):
                nc.sync.dma_start(out=xk_sb[:, d, :], in_=xkT_r[:, d, :])
            wv_sl[0] = wvp.tile([128, NT_DKV, 512], BF16, tag="wv", name="wv_0")
            nc.sync.dma_start(out=wv_sl[0][:, 0:4, :], in_=wvT_r[:, 0:4, 0:512])
            for d in range(4):
                nc.sync.dma_start(out=xv_sb[:, d, :], in_=xvT_r[:, d, :])
            nc.sync.dma_start(out=wv_sl[0][:, 4:8, :], in_=wvT_r[:, 4:8, 0:512])
            for d in range(4, NT_DKV):
                nc.sync.dma_start(out=xv_sb[:, d, :], in_=xvT_r[:, d, :])
            wq_sl[0] = wqp.tile([128, NT_DQ, 256], BF16, tag="wq", name="wq2_0")
            nc.sync.dma_start(out=wq_sl[0], in_=wqT_r[:, :, 0:256])
            for d in range(NT_DQ):
                nc.sync.dma_start(out=xq_sb[:, d, :], in_=xqT_r[:, d, :])
            load_wqk2(1, eng=nc.sync)
            load_wv(1, eng=nc.sync)
            # prefetch first out-projection weight tiles (parked until the
            # epilogue; also throttles the gpsimd wo stream via pool slots)
            wo_pre = []
            for et in range(8):
                wo_t = wop.tile([128, 512], BF16, tag="wo", name=f"wo_0_{et}")
                nc.sync.dma_start(
                    out=wo_t, in_=woT.ap()[et * 128:(et + 1) * 128, 0:512])
                wo_pre.append(wo_t)

            qT = {}
            kT = {}
            v4 = {}
            _CARRY = {}

            def qproj(h):
                """qT[h] [e128, s] <- sum_d wq-block.T @ xq (2 psum chunks)."""
                sl = wq_sl[h // 2]
                hoff = (h % 2) * 128
                qT[h] = qkp.tile([128, S], BF16, tag="qT", name=f"qT_{h}")
                for c in range(2):
                    ps = paps.tile([128, 512], F32, tag="pa", name=f"qps_{h}_{c}")
                    for d in range(NT_DQ):
                        nc.tensor.matmul(
                            ps,
                            sl[:, d, hoff:hoff + 128],
                            xq_sb[:, d, c * 512:(c + 1) * 512],
                            start=(d == 0),
                            stop=(d == NT_DQ - 1),
                        )
                    nc.scalar.activation(
                        out=qT[h][:, c * 512:(c + 1) * 512], in_=ps,
                        func=ACT_IDENT, bias=bq_sb[:, h:h + 1], scale=1.0)

            def kproj(h):
                sl = wk_sl[h // 2]
                hoff = (h % 2) * 128
                kT[h] = qkp.tile([128, S], BF16, tag="kT", name=f"kT_{h}")
                for c in range(2):
                    ps = paps.tile([128, 512], F32, tag="pa", name=f"kps_{h}_{c}")
                    for d in range(NT_DKV):
                        nc.tensor.matmul(
                            ps,
                            sl[:, d, hoff:hoff + 128],
                            xk_sb[:, d, c * 512:(c + 1) * 512],
                            start=(d == 0),
                            stop=(d == NT_DKV - 1),
                        )
                    nc.scalar.activation(
                        out=kT[h][:, c * 512:(c + 1) * 512], in_=ps,
                        func=ACT_IDENT, bias=bk_sb[:, h:h + 1], scale=1.0)

            def v4_alloc(g):
                v4[g] = v4p.tile([128, NT_S, 4 * VROW], BF16, tag="v4",
                                 name=f"v4_{g}")
                nc.vector.memset(
                    v4[g].rearrange("p t (h c) -> p t h c", c=VROW)
                    [:, :, :, HD:], 1.0)

            def vproj_part(g, st0, nst):
                """v4[g][:, st, :] <- (xv st-block).T @ wv-group, s-tiles
                st0..st0+nst-1; heads 4g..4g+3 with ones columns."""
                sl = wv_sl[g]
                for st in range(st0, st0 + nst):
                    ps = paps.tile([128, 512], F32, tag="pa",
                                   name=f"vps_{g}_{st}")
                    for d in range(NT_DKV):
                        nc.tensor.matmul(
                            ps,
                            xv_sb[:, d, st * 128:(st + 1) * 128],
                            sl[:, d, :],
                            start=(d == 0),
                            stop=(d == NT_DKV - 1),
                        )
                    nc.vector.tensor_copy(
                        out=v4[g][:, st, :].rearrange(
                            "p (h c) -> p h c", c=VROW)[:, :, 0:HD],
                        in_=ps.rearrange("p (h c) -> p h c", c=HD),
                    )

            PT = {}

            def scores_pair(h, s0):
                """two score tiles s0, s0+1: scoresT[sk,sq] -> exp -> pt."""
                for sk in (s0, s0 + 1):
                    pt_sk = pts.tile([128, S], BF16, tag="pt",
                                     name=f"pt{h}_{sk}")
                    PT.setdefault(h, {})[sk] = pt_sk
                    ss = ssps.tile([128, S], F32, tag="ss",
                                   name=f"ss_{h}_{sk}")
                    for c in range(2):
                        nc.tensor.matmul(
                            ss[:, c * 512:(c + 1) * 512],
                            kT[h][:, sk * 128:(sk + 1) * 128],
                            qT[h][:, c * 512:(c + 1) * 512],
                            start=True,
                            stop=True,
                        )
                    nc.scalar.activation(
                        out=pt_sk, in_=ss, func=ACT_EXP, bias=0.0, scale=1.0)

            def pv(h):
                pt_t = PT[h]
                g, hig = h // 4, h % 4
                ao_h = aohp.tile([128, NT_S, HD], BF16, tag="aoh",
                                 name=f"aoh_{h}")
                # 4 rotating accumulator slots: 2 op-pool tiles + both banks
                # of one ss tile (scores(h) has fully drained through exp by
                # now) — wide enough that the recip+scale drain never stalls
                # the PV matmul stream.
                ssa = ssps.tile([128, S], F32, tag="ss", name=f"pvss_{h}")
                opa = opps.tile([128, 512], F32, tag="op", name=f"opa_{h}")
                opb = opps.tile([128, 512], F32, tag="op", name=f"opb_{h}")
                # ss halves serve the EARLY sts so the borrowed ss slot
                # drains two st-groups sooner — it gates the next scores
                # rotation and, at head 15, the epilogue's first chunks
                chunk4 = [ssa[:, 0:512], ssa[:, 512:1024], opa, opb]
                for st in range(NT_S):
                    op = chunk4[st % 4]
                    for sk in range(NT_S):
                        nc.tensor.matmul(
                            op[:, 0:VROW],
                            pt_t[sk][:, st * 128:(st + 1) * 128],
                            v4[g][:, sk, hig * VROW:(hig + 1) * VROW],
                            start=(sk == 0),
                            stop=(sk == NT_S - 1),
                        )
                    rec = small.tile([128, 1], F32, tag="rec",
                                     name=f"rec_{h}_{st}")
                    nc.vector.reciprocal(out=rec, in_=op[:, HD:VROW])
                    nc.vector.tensor_scalar_mul(
                        ao_h[:, st, :], op[:, 0:HD], rec)
                nc.sync.dma_start_transpose(
                    out=aoT_sb[:, h, :].rearrange("p (t c) -> p t c", c=128),
                    in_=ao_h)

            # ---- prologue compute, d-outer interleaved so PE consumption
            # rate (4 mms per d-tile) stays behind the DMA arrival rate ----
            def prologue_qk2(proj_wsl, proj_x, nt_d, bias_sb, dst, nm):
                dst[0] = qkp.tile([128, S], BF16, tag=nm, name=f"{nm}_0")
                dst[1] = qkp.tile([128, S], BF16, tag=nm, name=f"{nm}_1")
                ps0 = [paps.tile([128, 512], F32, tag="pa",
                                 name=f"{nm}p0_{c}") for c in range(2)]
                sst = ssps.tile([128, S], F32, tag="ss", name=f"{nm}p1")
                ps1 = [sst[:, 0:512], sst[:, 512:1024]]
                for d in range(nt_d):
                    for hh, pss in ((0, ps0), (1, ps1)):
                        for c in range(2):
                            nc.tensor.matmul(
                                pss[c],
                                proj_wsl[:, d, hh * 128:(hh + 1) * 128],
                                proj_x[:, d, c * 512:(c + 1) * 512],
                                start=(d == 0),
                                stop=(d == nt_d - 1),
                            )
                for hh, pss in ((0, ps0), (1, ps1)):
                    for c in range(2):
                        nc.scalar.activation(
                            out=dst[hh][:, c * 512:(c + 1) * 512], in_=pss[c],
                            func=ACT_IDENT, bias=bias_sb[:, hh:hh + 1],
                            scale=1.0)

            prologue_qk2(wk_sl[0], xk_sb, NT_DKV, bk_sb, kT, "kT")
            v4_alloc(0)
            # vproj g0: two rounds of 4 s-tiles, d-outer (2 pa + 1 ss tile)
            for rnd in range(2):
                pv_ps = [paps.tile([128, 512], F32, tag="pa",
                                   name=f"vp{rnd}_{i}") for i in range(2)]
                sst = ssps.tile([128, S], F32, tag="ss", name=f"vp{rnd}ss")
                pv_ps.append(sst[:, 0:512])
                pv_ps.append(sst[:, 512:1024])
                for d in range(NT_DKV):
                    for i in range(4):
                        st = rnd * 4 + i
                        nc.tensor.matmul(
                            pv_ps[i],
                            xv_sb[:, d, st * 128:(st + 1) * 128],
                            wv_sl[0][:, d, :],
                            start=(d == 0),
                            stop=(d == NT_DKV - 1),
                        )
                for i in range(4):
                    st = rnd * 4 + i
                    nc.vector.tensor_copy(
                        out=v4[0][:, st, :].rearrange(
                            "p (h c) -> p h c", c=VROW)[:, :, 0:HD],
                        in_=pv_ps[i].rearrange("p (h c) -> p h c", c=HD),
                    )
            prologue_qk2(wq_sl[0], xq_sb, NT_DQ, bq_sb, qT, "qT")

            # ---- main loop over heads; iter h projects heads h+2 ----
            for h in range(H):
                # weight slice j covers heads 2j/2j+1, first needed in iter
                # 2j-2; load at iter 2j-3 (slot j-2 frees at iter 2j-5).
                if h % 2 == 1 and (h + 3) // 2 < H // 2:
                    load_wqk2((h + 3) // 2)
                # wv group g first needed in iter 4g-3 (wv0/wv1 in prologue).
                if h == 2:
                    load_wv(2)
                elif h == 6:
                    load_wv(3)

                # vproj for group g spread over iters 4g-4..4g-1, two
                # s-tiles per iter (g0 was done in the prologue).
                vg, vst0, vnst = None, 0, 0
                if h <= 11:
                    vg, vst0, vnst = h // 4 + 1, (h % 4) * 2, 2
                    if vst0 == 0:
                        v4_alloc(vg)

                if h < H - 2:
                    scores_pair(h, 0)
                    qproj(h + 2)
                    scores_pair(h, 2)
                    kproj(h + 2)
                    scores_pair(h, 4)
                    if vnst:
                        vproj_part(vg, vst0, vnst)
                    scores_pair(h, 6)
                    if h == 13:
                        # head 14's first score pair computed early so its
                        # exps drain on ACT during this iteration's slack
                        scores_pair(14, 0)
                else:
                    # iters 14/15 have no projection work to hide exp()
                    # under, so pull in out-projection accumulation for
                    # chunks st=4,5 (e2c=0) using the idle pa psum tiles
                    # and separately-streamed Wo tiles (et 0..13).
                    if h == H - 2:
                        opull = [paps.tile([128, 512], F32, tag="pa",
                                           name=f"opull_{i}")
                                 for i in range(2)]
                        _CARRY["opull"] = opull
                    opull = _CARRY["opull"]

                    def opull_ets(e0, e1):
                        # uses the wo_pre tiles parked since the prologue
                        for et in range(e0, e1):
                            for i in range(2):
                                nc.tensor.matmul(
                                    opull[i],
                                    aoT_sb[:, et, (4 + i) * 128:(5 + i) * 128],
                                    wo_pre[et],
                                    start=(et == 0),
                                    stop=False,
                                )

                    # score pair 0 was computed in the previous iteration;
                    # head 15's pair 0 is pulled into iter 14 here.
                    base = 0 if h == H - 2 else 4
                    scores_pair(h, 2)
                    opull_ets(base, base + 1)
                    scores_pair(h, 4)
                    opull_ets(base + 1, base + 2)
                    scores_pair(h, 6)
                    opull_ets(base + 2, base + 3)
                    if h == H - 2:
                        scores_pair(15, 0)
                    opull_ets(base + 3, base + 4)
                pv(h)

            # ---- output projection: out[s, e2] = aoT.T @ WoT ----
            # 4 column passes of 8 chunks; Wo streamed per (e-tile, pass).
            for e2c in range(4):
                chunks = []
                for i in range(2):
                    t = ssps.tile([128, S], F32, tag="ss", name=f"oss_{e2c}_{i}")
                    chunks.append(t[:, 0:512])
                    chunks.append(t[:, 512:1024])
                for i in range(2):
                    if e2c == 0:
                        chunks.append(_CARRY["opull"][i])
                    else:
                        chunks.append(paps.tile([128, 512], F32, tag="pa",
                                                name=f"opa_{e2c}_{i}"))
                for i in range(2):
                    chunks.append(opps.tile([128, 512], F32, tag="op",
                                            name=f"oop_{e2c}_{i}"))
                def get_wo(et):
                    if e2c == 0 and et < len(wo_pre):
                        return wo_pre[et]
                    wo_t = wop.tile([128, 512], BF16, tag="wo",
                                    name=f"wo_{e2c}_{et}")
                    nc.gpsimd.dma_start(
                        out=wo_t,
                        in_=woT.ap()[et * 128:(et + 1) * 128,
                                     e2c * 512:(e2c + 1) * 512])
                    return wo_t

                # et-major streaming phase (et 0..7)
                for et in range(8):
                    wo_t = get_wo(et)
                    for st in range(NT_S):
                        if e2c == 0 and st in (4, 5):
                            continue  # accumulated during iters 14/15
                        nc.tensor.matmul(
                            chunks[st],
                            aoT_sb[:, et, st * 128:(st + 1) * 128],
                            wo_t,
                            start=(et == 0),
                            stop=False,
                        )
                # staggered tail: each chunk finishes its last 8 ets, then
                # copy+store immediately so completions pipeline out at a
                # spacing (~1.7us) above the copy+DMA drain rate
                wo_tail = {et: get_wo(et) for et in range(8, NT_E)}
                for st in range(NT_S):
                    for et in range(8, NT_E):
                        nc.tensor.matmul(
                            chunks[st],
                            aoT_sb[:, et, st * 128:(st + 1) * 128],
                            wo_tail[et],
                            start=False,
                            stop=(et == NT_E - 1),
                        )
                    ot = outsb.tile([128, 512], F32, tag="outt",
                                    name=f"ot_{e2c}_{st}")
                    if st % 2 == 0:
                        nc.vector.tensor_copy(out=ot, in_=chunks[st])
                        eng = nc.sync
                    else:
                        nc.scalar.copy(out=ot, in_=chunks[st])
                        eng = nc.scalar
                    eng.dma_start(
                        out=out.ap()[st * 128:(st + 1) * 128,
                                     e2c * 512:(e2c + 1) * 512],
                        in_=ot,
                    )

    nc.compile()
    return nc


def _get_nc():
    if "nc" not in _CACHED:
        _CACHED["nc"] = _build()
    return _CACHED["nc"]


def _numpy_reference(query, key, value, attention_mask,
                     Wq, bq, Wk, bk, Wv, bv, Wo, bo):
    # general fallback (only used when attention_mask isn't all ones)
    Bb, SQ, _ = query.shape
    SK = key.shape[1]
    q = query @ Wq.T + bq
    k = key @ Wk.T + bk
    v = value @ Wv.T + bv
    q = q.reshape(Bb, SQ, H, HD).transpose(0, 2, 1, 3)
    k = k.reshape(Bb, SK, H, HD).transpose(0, 2, 1, 3)
    v = v.reshape(Bb, SK, H, HD).transpose(0, 2, 1, 3)
    scores = np.einsum("bhqd,bhkd->bhqk", q, k) * (HD ** -0.5)
    scores = np.where(attention_mask[:, None, :, :] == 0,
                      np.float32(-1e10), scores)
    scores -= scores.max(-1, keepdims=True)
    p = np.exp(scores)
    p /= p.sum(-1, keepdims=True)
    o = np.einsum("bhqk,bhkd->bhqd", p, v)
    o = o.transpose(0, 2, 1, 3).reshape(Bb, SQ, E)
    return (o @ Wo.T + bo).astype(np.float32)


def _prepare_in_maps(inputs):
    query = np.asarray(inputs["query"], dtype=np.float32)
    key = np.asarray(inputs["key"], dtype=np.float32)
    value = np.asarray(inputs["value"], dtype=np.float32)
    Wq = np.asarray(inputs["Wq"], dtype=np.float32)
    bq = np.asarray(inputs["bq"], dtype=np.float32)
    Wk = np.asarray(inputs["Wk"], dtype=np.float32)
    bk = np.asarray(inputs["bk"], dtype=np.float32)
    Wv = np.asarray(inputs["Wv"], dtype=np.float32)
    Wo = np.asarray(inputs["Wo"], dtype=np.float32)

    scale = np.float32(HD ** -0.5)
    bf = ml_dtypes.bfloat16
    wqT = np.ascontiguousarray((Wq.T * scale).astype(bf))
    wkT = np.ascontiguousarray(Wk.T.astype(bf))
    wvT = np.ascontiguousarray(Wv.T.astype(bf))
    woT = np.ascontiguousarray(Wo.T.astype(bf))
    bq_s = (bq * scale).astype(np.float32)

    in_maps = []
    for b in range(B):
        in_maps.append({
            "xqT": np.ascontiguousarray(query[b].T.astype(bf)),
            "xkT": np.ascontiguousarray(key[b].T.astype(bf)),
            "xvT": np.ascontiguousarray(value[b].T.astype(bf)),
            "wqT": wqT, "wkT": wkT, "wvT": wvT, "woT": woT,
            "bq": bq_s, "bk": bk.astype(np.float32),
        })
    return in_maps


def run_on_device(inputs, **spmd_kwargs):
    """Run the bass kernel; returns (out [B,S,E] f32, BassKernelResults)."""
    in_maps = _prepare_in_maps(inputs)
    Wo = np.asarray(inputs["Wo"], dtype=np.float64)
    bv = np.asarray(inputs["bv"], dtype=np.float64)
    bo = np.asarray(inputs["bo"], dtype=np.float64)
    bo_eff = (Wo @ bv + bo).astype(np.float32)
    res = run_bass_kernel_spmd(_get_nc(), in_maps,
                               core_ids=list(range(B)), **spmd_kwargs)
    out = np.stack([res.results[b]["out"] for b in range(B)], axis=0)
    return (out + bo_eff).astype(np.float32), res


def kernel(**inputs):
    mask = np.asarray(inputs["attention_mask"])
    if not mask.all():
        return _numpy_reference(
            np.asarray(inputs["query"], dtype=np.float32),
            np.asarray(inputs["key"], dtype=np.float32),
            np.asarray(inputs["value"], dtype=np.float32), mask,
            np.asarray(inputs["Wq"], dtype=np.float32),
            np.asarray(inputs["bq"], dtype=np.float32),
            np.asarray(inputs["Wk"], dtype=np.float32),
            np.asarray(inputs["bk"], dtype=np.float32),
            np.asarray(inputs["Wv"], dtype=np.float32),
            np.asarray(inputs["bv"], dtype=np.float32),
            np.asarray(inputs["Wo"], dtype=np.float32),
            np.asarray(inputs["bo"], dtype=np.float32))
    out, _ = run_on_device(inputs)
    return out
